# revision 27
# baseline (speedup 1.0000x reference)
"""Trainium2 Bass kernel for nn_New3_77395310674432 (sparse_attention).

Pipeline (8-core SPMD, one NEFF, NO collectives), default "linear" path:
  A') region via linearized softmax: scores s = q@k.T/16 are provably tiny
      (|s| <= max||q||^2/16 ~ 2.3e-3 for these inputs; the Cauchy-Schwarz
      bound is checked on the host and the exact kernel below is used
      instead if it exceeds SCORE_BOUND). exp(s) ~= 1+s  =>
      region = (sq + q C/16) / (N + q sk/16),  C = K^T Q  [256,256],
      by matmul associativity (QK^T)Q = Q(K^TQ). Computed directly in the
      transposed layout rgT on every core (replicated, ~2.6 GFLOP total)
      -- no N^2 work, no AllGather, no DMA transpose.
  A'+C interleaved: per-core full item tables (factorized projections)
      tabK = feats@Wk.T (384) | tabG = feats@M (256; M = Wv.T@tgt.T for
      this core's 256 targets), written merged per 128-item chunk right
      after its region block; feats = [emb_item | region]. Target B-rows
      (Wq/Wc/bv projections, incl. the reshape-quirk key-bias fold) are
      computed directly from the transposed targets -- no 10k-row B table.
  D) Per-core batch shard (128/core): per-l indirect gathers of merged
      K|G rows by user indices (HW honors one index per partition per
      call), s0 accumulated per gather chunk via the reshape-quirk d-block
      decomposition (flat positions d*100+l of 96 d's == 25 table rows),
      u0 via diag-extract of G, exp/mask/pow(beta=.5) -> predictions.

Perf notes (2026-08-11 session, HW NTFF traces via antenv.axon_hooks shim):
  - s0 multiply reads kec j-flat CONTIGUOUS (d-major [96,100] view); the
    old scrambled [l,d] view ran 2.2 ns/elem on HW vs ~1.05 contiguous.
  - s0/u0 reduces: bf16 fold halves (DVE 2x mode needs every operand's
    LAST ap dim packed >=2 and 2-byte) then one small strided f32 reduce.
  - 1/x via reciprocal_approx_fast (InstReciprocal was 7.8 ns/elem on HW).
  - Copies deliberately on Scalar engine (Act): they pipeline against
    DVE across gather chunks; moving them to DVE regressed (-19%).
  - InstTensorTensorReduce and gpsimd.dma_gather (InstDMAGatherAnt) both
    FAIL on this axon/PJRT runtime (INTERNAL at execute) -- do not use.
    The 100 indirect gathers cost ~1.1us each on GpSimd (descriptor gen),
    ~110us/core total; dma_gather would have cut that to ~8us.
  - Single-shot NTFF exec times vary +-19% with clock/pstate; only the
    repeat-slope (R=1 vs 16) numbers are comparable across builds.

Exact fallback (_build_exact): the original full-softmax kernel (row-
sharded N^2 attention + AllGather), used only if the score bound fails.
"""
import sys
if "/opt/trn_rl_repo" not in sys.path:
    sys.path.insert(0, "/opt/trn_rl_repo")
import numpy as np
import ml_dtypes

bf16 = ml_dtypes.bfloat16

N_ITEMS = 10000
D = 128
D2 = 256
D3 = 384
B = 1024
L = 100
NCORES = 8
BSH = B // NCORES            # 128 batches per core
NPAD = 79 * 128              # 10112 padded items
NCH = 79                     # 128-row chunks
TABB_W = 512                 # [Q0 384 | Crow 100 | bvdot 1 | pad 27]
PEN = -1.0e9
NFCH = 512                   # free-dim chunk for the region matmuls
SCORE_BOUND = 0.05           # |s| above this -> exact fallback

NSH = N_ITEMS // NCORES     # exact-path row shard per core
MBLOCKS = [(0, 512), (512, 512), (1024, 226)]  # exact-path stage-A m-blocks
USE_DMA_GATHER = False      # InstDMAGatherAnt fails on this runtime (HW)




def _build_linear(repeat=1, phases="ACD"):
    import concourse.bass as bass
    import concourse.tile as tile
    from concourse import bacc, mybir
    from concourse.masks import make_identity

    F32 = mybir.dt.float32
    BF = mybir.dt.bfloat16
    I32 = mybir.dt.int32
    MUL = mybir.AluOpType.mult
    ADD = mybir.AluOpType.add

    nc = bacc.Bacc("TRN2", target_bir_lowering=False, debug=False,
                   num_devices=NCORES)

    def din(name, shape, dt):
        return nc.dram_tensor(name, shape, dt, kind="ExternalInput").ap()

    qt2_d = din("qt2", [2, 128, NPAD], BF)          # q^T, zero-padded
    qe_d = din("qe", [NCH, 128, D2 + 1], BF)        # [q | 1] rows, zero-padded
    embT_d = din("embT", [128, NPAD], BF)
    embg_d = din("embg", [N_ITEMS, D], BF)
    rhsK_d = din("rhsK", [3, 128, D3], BF)
    rhsB_d = din("rhsB", [3, 128, TABB_W], BF)
    wv3_d = din("wv3", [3, 128, D3], BF)
    consts_d = din("consts", [1, D3 + L], BF)
    sqr_d = din("sqr", [1, D2 + 1], F32)            # [sum_n q | N]
    sqc_d = din("sqc", [128, 2], F32)               # sq as column per j-half
    user_d = din("user", [BSH, L], I32)
    userg_d = din("userg", [128, L * BSH // 16], mybir.dt.int16)
    item_d = din("item", [BSH, 2], I32)
    pred_d = nc.dram_tensor("pred", [BSH, 4], F32, kind="ExternalOutput").ap()

    nch_sl = []
    n0 = 0
    while n0 < NPAD:
        w = min(NFCH, NPAD - n0)
        nch_sl.append((n0, w))
        n0 += w

    with tile.TileContext(nc) as tc:
        with (
            tc.tile_pool(name="persist", bufs=1) as pp,
            tc.tile_pool(name="dram", bufs=1, space="DRAM") as dr,
        ):
            tabKG = dr.tile([NPAD, D3 + D2], BF)

            ident = pp.tile([128, 128], BF)
            make_identity(nc, ident[:])
            user_t = pp.tile([BSH, L], I32)
            nc.sync.dma_start(user_t[:], user_d[:])
            userg_t = pp.tile([128, L * BSH // 16], mybir.dt.int16)
            nc.sync.dma_start(userg_t[:], userg_d[:])
            item_t = pp.tile([BSH, 2], I32)
            nc.sync.dma_start(item_t[:], item_d[:])
            crow = pp.tile([1, D3 + L], BF)
            nc.sync.dma_start(crow[:], consts_d[:])
            crep = pp.tile([128, D3 + L], BF)
            nc.gpsimd.partition_broadcast(crep[:], crow[:])
            ident2 = pp.tile([128, 2 * BSH], BF)
            nc.vector.tensor_copy(ident2[:, 0:BSH], ident[:])
            nc.vector.tensor_copy(ident2[:, BSH:2 * BSH], ident[:])

            # ---------------- Phase A': linearized region ----------------
            def _phase_a(rep, par):
                rgT = par["rgT"]
                regtg = par["regtg"]
                bi = par["bi"]
                with (
                    tc.tile_pool(name=f"pa{rep}", bufs=1) as pa,
                    tc.tile_pool(name=f"pa_w{rep}", bufs=3) as pw,
                    tc.tile_pool(name=f"pa_cw{rep}", bufs=3) as pcw,
                    tc.tile_pool(name=f"pa_pr{rep}", bufs=1, space="PSUM") as ppr,
                ):
                    # qe first, quartered: C16 accumulation starts on the
                    # first quarter while the rest (and qt2/embT) stream in.
                    qe_sb = pa.tile([128, NCH, D2 + 1], BF)
                    qcuts = [0, 6, 16, 32, 48, 64, NCH]
                    for qi in range(len(qcuts) - 1):
                        nc.sync.dma_start(
                            qe_sb[:, qcuts[qi]:qcuts[qi + 1], :],
                            qe_d[qcuts[qi]:qcuts[qi + 1]].rearrange(
                                "c p w -> p c w"))
                    qt2_sb = pa.tile([128, 2, NPAD], BF)
                    nc.sync.dma_start(qt2_sb[:], qt2_d[:].rearrange("c p n -> p c n"))
                    sqrow = pa.tile([1, D2 + 1], F32)
                    nc.sync.dma_start(sqrow[:], sqr_d[:])
                    sqrep = pa.tile([128, D2 + 1], F32)
                    nc.gpsimd.partition_broadcast(sqrep[:], sqrow[:])
                    sqbf = pa.tile([1, D2], BF)
                    nc.vector.tensor_copy(sqbf[:], sqrow[:, 0:D2])
                    ones_row = pa.tile([1, NFCH], BF)
                    nc.gpsimd.memset(ones_row[:], 1.0)
                    et_sb = pa.tile([128, NPAD], BF)
                    nc.sync.dma_start(et_sb[:], embT_d[:])
                    rk_sb = pa.tile([128, 3, D3], BF)
                    nc.sync.dma_start(rk_sb[:], rhsK_d[:].rearrange("c p w -> p c w"))
                    rb_sb = pa.tile([128, 3, TABB_W], BF)
                    nc.sync.dma_start(rb_sb[:], rhsB_d[:].rearrange("c p w -> p c w"))
                    wv_sb = pa.tile([128, 3, D3], BF)
                    nc.sync.dma_start(wv_sb[:], wv3_d[:].rearrange("c p w -> p c w"))

                    # C_ext[i, j] = sum_n k[n,i] q[n,j]  (+ sk in col 256)
                    psC = [ppr.tile([128, D2 + 1], F32, tag=f"psC{i}",
                                    name=f"psC{i}_{rep}") for i in range(2)]
                    for ci in range(NCH):
                        st, sp = (ci == 0), (ci == NCH - 1)
                        nc.tensor.matmul(psC[0][:], qe_sb[:, ci, 128:256],
                                         qe_sb[:, ci, :], start=st, stop=sp)
                        nc.tensor.matmul(psC[1][:], qe_sb[:, ci, 0:128],
                                         qe_sb[:, ci, :], start=st, stop=sp)
                    C16 = pa.tile([128, 2, D2 + 1], BF)
                    nc.scalar.mul(C16[:, 0, :], psC[0][:], 1.0 / 16.0)
                    nc.scalar.mul(C16[:, 1, :], psC[1][:], 1.0 / 16.0)

                    # per-target region rows + tcT/grhs/bi (psum pool scoped)
                    tgt = pa.tile([128, 2, D3], BF)
                    tcT = pa.tile([128, 3, 2 * BSH], BF)
                    grhs = pa.tile([128, 3, 2 * BSH], BF)
                    with tc.tile_pool(name=f"pa_pt{rep}", bufs=1,
                                      space="PSUM") as ppst:
                        for s in range(2):
                            qg = pa.tile([128, D2 + 1], BF, tag=f"qg{s}")
                            nc.gpsimd.indirect_dma_start(
                                out=qg[:], out_offset=None,
                                in_=qe_d[:].rearrange("c p w -> (c p) w"),
                                in_offset=bass.IndirectOffsetOnAxis(
                                    ap=item_t[:, s:s + 1], axis=0))
                            qgT = pa.tile([128, 2, 128], BF, tag=f"qgT{s}")
                            for ic in range(2):
                                pstr = ppst.tile([128, 128], BF, tag="pstr_sq")
                                nc.tensor.transpose(
                                    pstr[:], qg[:, ic * 128:(ic + 1) * 128],
                                    ident[:])
                                nc.scalar.copy(qgT[:, ic, :], pstr[:])
                            psT = ppst.tile([128, D2 + 1], F32, tag="psT")
                            nc.tensor.matmul(psT[:], qgT[:, 0, :], C16[:, 0, :],
                                             start=True, stop=False)
                            nc.tensor.matmul(psT[:], qgT[:, 1, :], C16[:, 1, :],
                                             start=False, stop=True)
                            tgta = pw.tile([128, D2 + 1], F32, tag="tgta")
                            nc.vector.tensor_tensor(out=tgta[:], in0=psT[:],
                                                    in1=sqrep[:], op=ADD)
                            rdent = pw.tile([128, 1], F32, tag="rdent")
                            nc.vector.reciprocal_approx_fast(
                                rdent[:], tgta[:, D2:D2 + 1])
                            nc.vector.tensor_scalar_mul(
                                regtg[:, s, :], tgta[:, 0:D2], rdent[:])
                            # targets: [emb | region] rows
                            nc.gpsimd.indirect_dma_start(
                                out=tgt[:, s, 0:D], out_offset=None,
                                in_=embg_d[:],
                                in_offset=bass.IndirectOffsetOnAxis(
                                    ap=item_t[:, s:s + 1], axis=0))
                            nc.scalar.copy(tgt[:, s, D:D3], regtg[:, s, :])
                        # transpose targets -> tcT [feat, (pos128|neg128)]
                        for oc in range(3):
                            for s in range(2):
                                pstr = ppst.tile([128, 128], BF, tag="pstr_sq")
                                nc.tensor.transpose(
                                    pstr[:], tgt[:, s, oc * 128:(oc + 1) * 128],
                                    ident[:])
                                nc.scalar.copy(
                                    tcT[:, oc, s * BSH:(s + 1) * BSH], pstr[:])
                        # M[in, tgt] = sum_out Wv[out, in] * tcT[out, tgt]
                        for ic in range(3):
                            psM = ppst.tile([128, 2 * BSH], F32, tag="psT")
                            for oc in range(3):
                                nc.tensor.matmul(
                                    psM[:], wv_sb[:, oc, ic * 128:(ic + 1) * 128],
                                    tcT[:, oc, :], start=(oc == 0),
                                    stop=(oc == 2))
                            nc.vector.tensor_copy(grhs[:, ic, :], psM[:])
                        # target B-rows directly from tcT (no 10k-item table)
                        for s in range(2):
                            psTB = ppst.tile([128, TABB_W], F32, tag="psTB")
                            for oc in range(3):
                                nc.tensor.matmul(
                                    psTB[:], tcT[:, oc, s * BSH:(s + 1) * BSH],
                                    rb_sb[:, oc, :], start=(oc == 0),
                                    stop=(oc == 2))
                            nc.scalar.copy(bi[:, s, :], psTB[:])

                    # interleaved: region chunk n, then table chunks 4n..4n+3
                    reg_ps = tc.tile_pool(name=f"pa_rg{rep}", bufs=1,
                                          space="PSUM")
                    ppsr = reg_ps.__enter__()
                    for nci, (n0, w) in enumerate(nch_sl):
                        sl = slice(n0, n0 + w)
                        psD = ppsr.tile([1, NFCH], F32, tag="psD", bufs=1)
                        nc.tensor.matmul(psD[:, :w], C16[:, 0, 256:257],
                                         qt2_sb[:, 0, sl], start=True, stop=False)
                        nc.tensor.matmul(psD[:, :w], C16[:, 1, 256:257],
                                         qt2_sb[:, 1, sl], start=False, stop=True)
                        drec = pw.tile([1, NFCH], F32, tag="drec")
                        nc.vector.tensor_scalar_add(drec[:, :w], psD[:, :w],
                                                    float(N_ITEMS))
                        nc.vector.reciprocal_approx_fast(drec[:, :w],
                                                         drec[:, :w])
                        drecb = pw.tile([128, NFCH], F32, tag="drecb")
                        nc.gpsimd.partition_broadcast(drecb[:, :w],
                                                      drec[:, :w])
                        psNT2 = ppsr.tile([128, 2, NFCH], F32, tag="psNT",
                                          bufs=1)
                        for jh in range(2):
                            nc.tensor.matmul(
                                psNT2[:, jh, :w],
                                C16[:, 0, jh * 128:(jh + 1) * 128],
                                qt2_sb[:, 0, sl], start=True, stop=False)
                            nc.tensor.matmul(
                                psNT2[:, jh, :w],
                                C16[:, 1, jh * 128:(jh + 1) * 128],
                                qt2_sb[:, 1, sl], start=False, stop=False)
                            nc.tensor.matmul(
                                psNT2[:, jh, :w],
                                sqbf[0:1, jh * 128:(jh + 1) * 128],
                                ones_row[0:1, :w], start=False, stop=True)
                        nc.vector.tensor_tensor(
                            out=rgT[:, :, sl], in0=psNT2[:, :, :w],
                            in1=drecb[:, :w].unsqueeze(1).to_broadcast(
                                [128, 2, w]),
                            op=MUL)
                        # table chunks covered by this region block
                        for ch in range(4 * nci, min(4 * nci + 4, NCH)):
                            tsl = slice(ch * 128, (ch + 1) * 128)
                            psK = ppsr.tile([128, D3], F32, tag="psK")
                            psG = ppsr.tile([128, D2], F32, tag="psG")
                            for j in range(3):
                                lh = (et_sb[:, tsl] if j == 0
                                      else rgT[:, j - 1, tsl])
                                nc.tensor.matmul(psK[:], lh, rk_sb[:, j, :],
                                                 start=(j == 0), stop=(j == 2))
                                nc.tensor.matmul(psG[:], lh, grhs[:, j, :],
                                                 start=(j == 0), stop=(j == 2))
                            cKG = pcw.tile([128, D3 + D2], BF, tag="cKG")
                            nc.scalar.copy(cKG[:, 0:D3], psK[:])
                            nc.scalar.copy(cKG[:, D3:D3 + D2], psG[:])
                            nc.sync.dma_start(tabKG[tsl, :], cKG[:])
                    reg_ps.__exit__(None, None, None)

            def _phase_c(rep, par):
                pass

            # ---------------- Phase G: gathers only (bench probe) ------
            def _phase_g(rep, par):
                with (
                    tc.tile_pool(name=f"pg{rep}", bufs=1) as pg,
                    tc.tile_pool(name=f"pg_w{rep}", bufs=2) as pgw,
                ):
                    preds = pg.tile([128, 2], F32)
                    LCH = 25
                    for dc in range(L // LCH):
                        l0 = dc * LCH
                        kg = pgw.tile([128, LCH, D3 + D2], BF, tag="kg")
                        for l in range(l0, l0 + LCH):
                            nc.gpsimd.indirect_dma_start(
                                out=kg[:, l - l0, :], out_offset=None,
                                in_=tabKG[:],
                                in_offset=bass.IndirectOffsetOnAxis(
                                    ap=user_t[:, l:l + 1], axis=0))
                        nc.vector.tensor_copy(preds[:], kg[:, 0, 0:2])
                    nc.sync.dma_start(pred_d[:], preds[:])

            # ---------------- Phase D: attention_network ----------------
            def _phase_d(rep, par):
                bi = par["bi"]
                with (
                    tc.tile_pool(name=f"pd{rep}", bufs=1) as pd,
                    tc.tile_pool(name=f"pd_w{rep}", bufs=2) as pdw,
                ):
                    denb = pd.tile([128, 2], F32)
                    num_all = pd.tile([128, 2], F32)
                    bvd_all = pd.tile([128, 2], F32)

                    # qp first: s0 partials need it inside the gather loop
                    qp2 = pd.tile([128, 2, D3], BF)
                    for s in range(2):
                        nc.vector.tensor_tensor(out=qp2[:, s, :],
                                                in0=bi[:, s, 0:D3],
                                                in1=crep[:, 0:D3], op=ADD)

                    # merged K|G gather, l-chunked; per chunk: diag-extract G
                    # and accumulate the s0 d-chunk partial (the reshape quirk
                    # maps flat [d*100+l for 96 d's] onto exactly 25 ke rows).
                    u0i = pd.tile([128, L, 2], F32)
                    s0b = pd.tile([128, 2, L], F32)
                    LCH = 25
                    DCH = 96
                    NIDX = LCH * 128
                    for dc in range(L // LCH):
                        l0 = dc * LCH
                        kg = pdw.tile([128, LCH, D3 + D2], BF, tag="kg")
                        if USE_DMA_GATHER:
                            # one batched gather: row u[b,l0+c] -> kg[b,c,:]
                            # (idx i=c*128+b at userg[i%16, dc*200 + i//16])
                            nc.gpsimd.dma_gather(
                                kg[:], tabKG[:],
                                userg_t[:, dc * (NIDX // 16):
                                        (dc + 1) * (NIDX // 16)],
                                NIDX, NIDX, D3 + D2)
                        else:
                            for l in range(l0, l0 + LCH):
                                nc.gpsimd.indirect_dma_start(
                                    out=kg[:, l - l0, :], out_offset=None,
                                    in_=tabKG[:],
                                    in_offset=bass.IndirectOffsetOnAxis(
                                        ap=user_t[:, l:l + 1], axis=0))
                        kec = pdw.tile([128, LCH, D3], BF, tag="kec", bufs=2)
                        nc.scalar.copy(kec[:], kg[:, :, 0:D3])
                        prod = pdw.tile([128, LCH, D2], BF, tag="prod", bufs=1)
                        nc.vector.tensor_tensor(
                            out=prod[:],
                            in0=kg[:, :, D3:D3 + D2],
                            in1=ident2[:].unsqueeze(1).to_broadcast(
                                [128, LCH, D2]),
                            op=MUL)
                        # u0: fold 128-wide diag segments 128->16 in bf16
                        # (2x DVE mode: last AP dim packed), then f32 reduce.
                        pv = prod[:].rearrange("p a (s t) -> p a s t", s=2)
                        uf1 = pdw.tile([128, LCH, 2, 64], BF, tag="uf1", bufs=1)
                        nc.vector.tensor_tensor(
                            out=uf1[:], in0=pv[:, :, :, 0:64],
                            in1=pv[:, :, :, 64:128], op=ADD)
                        uf2 = pdw.tile([128, LCH, 2, 32], BF, tag="uf2", bufs=1)
                        nc.vector.tensor_tensor(
                            out=uf2[:], in0=uf1[:, :, :, 0:32],
                            in1=uf1[:, :, :, 32:64], op=ADD)
                        uf3 = pdw.tile([128, LCH, 2, 16], BF, tag="uf3", bufs=1)
                        nc.vector.tensor_tensor(
                            out=uf3[:], in0=uf2[:, :, :, 0:16],
                            in1=uf2[:, :, :, 16:32], op=ADD)
                        nc.vector.tensor_reduce(
                            u0i[:, l0:l0 + LCH, :].rearrange("p a b -> p (a b)"),
                            uf3[:].rearrange("p a s t -> p (a s) t"),
                            axis=mybir.AxisListType.X, op=ADD)
                        # zc2 in d-major layout, in two 48-d halves (halves
                        # the SBUF working set so kec can double-buffer):
                        # in0 = kec j-flat (contiguous, packed innermost) so
                        # the multiply avoids the HW strided-read penalty.
                        kec_dm = kec[:].rearrange("p a b -> p (a b)").rearrange(
                            "p (d l) -> p d l", l=L)  # [128, 96, 100] contig
                        DH = DCH // 2
                        for dh in range(2):
                            d0 = dc * DCH + dh * DH
                            zc2 = pdw.tile([128, 2, DH, L], BF, tag="zc",
                                           bufs=1)
                            nc.vector.tensor_tensor(
                                out=zc2[:],
                                in0=kec_dm[:, dh * DH:(dh + 1) * DH, :]
                                    .unsqueeze(1).to_broadcast([128, 2, DH, L]),
                                in1=qp2[:, :, d0:d0 + DH]
                                    .unsqueeze(3).to_broadcast([128, 2, DH, L]),
                                op=MUL)
                            # fold d 48 -> 6 in bf16 (2x mode, contiguous
                            # halves), then strided f32 reduce over d
                            zf1 = pdw.tile([128, 2, DH // 2, L], BF,
                                           tag="zf1", bufs=1)
                            nc.vector.tensor_tensor(
                                out=zf1[:], in0=zc2[:, :, 0:24, :],
                                in1=zc2[:, :, 24:48, :], op=ADD)
                            zf2 = pdw.tile([128, 2, DH // 4, L], BF,
                                           tag="zf2", bufs=1)
                            nc.vector.tensor_tensor(
                                out=zf2[:], in0=zf1[:, :, 0:12, :],
                                in1=zf1[:, :, 12:24, :], op=ADD)
                            zf3 = pdw.tile([128, 2, DH // 8, L], BF,
                                           tag="zf3", bufs=1)
                            nc.vector.tensor_tensor(
                                out=zf3[:], in0=zf2[:, :, 0:6, :],
                                in1=zf2[:, :, 6:12, :], op=ADD)
                            zf3v = zf3[:].rearrange("p s d l -> p s l d")
                            if dc == 0 and dh == 0:
                                nc.vector.tensor_reduce(
                                    s0b[:].rearrange("p a b -> p (a b)"), zf3v,
                                    axis=mybir.AxisListType.X, op=ADD)
                            else:
                                part = pdw.tile([128, 2, L], F32, tag="part")
                                nc.vector.tensor_reduce(
                                    part[:].rearrange("p a b -> p (a b)"),
                                    zf3v, axis=mybir.AxisListType.X, op=ADD)
                                nc.vector.tensor_tensor(
                                    out=s0b[:], in0=s0b[:], in1=part[:],
                                    op=ADD)

                    for s in range(2):
                        ct = pdw.tile([128, L], F32, tag="ct")
                        nc.vector.tensor_tensor(out=ct[:], in0=bi[:, s, D3:D3 + L],
                                                in1=crep[:, D3:D3 + L], op=ADD)
                        if s == 0:
                            eq = pdw.tile([128, L], F32, tag="eq")
                            nc.vector.tensor_tensor(
                                out=eq[:], in0=user_t[:],
                                in1=item_t[:, 0:1].to_broadcast([BSH, L]),
                                op=mybir.AluOpType.is_equal)
                            pen = pdw.tile([128, L], F32, tag="pen")
                            nc.vector.tensor_scalar_mul(pen[:], eq[:], PEN)
                            nc.vector.tensor_tensor(out=ct[:], in0=ct[:],
                                                    in1=pen[:], op=ADD)
                        nc.vector.tensor_tensor(out=ct[:], in0=s0b[:, s, :],
                                                in1=ct[:], op=ADD)
                        expa = pdw.tile([128, L], F32, tag="expa")
                        nc.scalar.activation(
                            expa[:], ct[:], mybir.ActivationFunctionType.Exp,
                            scale=float(1.0 / np.sqrt(D3)),
                            accum_out=denb[:, s:s + 1])
                        wu = pdw.tile([128, L], F32, tag="wu")
                        nc.vector.tensor_tensor(out=wu[:], in0=expa[:],
                                                in1=u0i[:, :, s], op=MUL)
                        nc.vector.tensor_reduce(num_all[:, s:s + 1], wu[:],
                                                axis=mybir.AxisListType.X, op=ADD)
                        nc.vector.tensor_copy(bvd_all[:, s:s + 1],
                                              bi[:, s, D3 + L:D3 + L + 1])

                    # device outputs [num + bvd*S | S]; host finishes
                    # pred = (num + bvd*S)/sqrt(S) -- keeps Exp as the only
                    # ACT table function (no per-rep table reloads)
                    t2 = pd.tile([128, 2], F32)
                    nc.vector.tensor_tensor(out=t2[:], in0=bvd_all[:],
                                            in1=denb[:], op=MUL)
                    preds4 = pd.tile([128, 4], F32)
                    nc.vector.tensor_tensor(out=preds4[:, 0:2], in0=num_all[:],
                                            in1=t2[:], op=ADD)
                    nc.vector.tensor_copy(preds4[:, 2:4], denb[:])
                    nc.sync.dma_start(pred_d[:], preds4[:])

            for rep in range(repeat):
                with tc.tile_pool(name=f"pard{rep}", bufs=1) as pardp:
                    par = {
                        "bi": pardp.tile([128, 2, TABB_W], BF,
                                         name=f"bi_{rep}"),
                    }
                    with tc.tile_pool(name=f"parc{rep}", bufs=1) as parcp:
                        par["rgT"] = parcp.tile([128, 2, NPAD], BF,
                                                name=f"rgT_{rep}")
                        par["regtg"] = parcp.tile([128, 2, D2], BF,
                                                  name=f"regtg_{rep}")
                        if "A" in phases:
                            _phase_a(rep, par)
                        if "C" in phases:
                            _phase_c(rep, par)
                    if "G" in phases:
                        _phase_g(rep, par)
                    if "D" in phases:
                        _phase_d(rep, par)

    nc.compile()
    return nc


def _prep_linear(inputs):
    f = np.float32
    user = np.asarray(inputs["user"]).astype(np.int32)
    item_i = np.asarray(inputs["item_i"]).astype(np.int32)
    item_j = np.asarray(inputs["item_j"]).astype(np.int32)
    emb_item = np.asarray(inputs["emb_item"], dtype=f)
    emb_in = np.asarray(inputs["emb_in"], dtype=f)
    emb_out = np.asarray(inputs["emb_out"], dtype=f)
    Wq = np.asarray(inputs["Wq"], dtype=f)
    bq = np.asarray(inputs["bq"], dtype=f)
    Wk = np.asarray(inputs["Wk"], dtype=f)
    bk = np.asarray(inputs["bk"], dtype=f)
    Wv = np.asarray(inputs["Wv"], dtype=f)
    bv = np.asarray(inputs["bv"], dtype=f)

    q = np.concatenate([emb_in, emb_out], 1)            # [N, 256]
    qt2 = np.zeros((D2, NPAD), f)
    qt2[:, :N_ITEMS] = q.T
    qt2 = qt2.reshape(2, 128, NPAD).astype(bf16)
    qe = np.zeros((NPAD, D2 + 1), f)
    qe[:N_ITEMS, :D2] = q
    qe[:N_ITEMS, D2] = 1.0
    qe = qe.reshape(NCH, 128, D2 + 1).astype(bf16)
    embT = np.zeros((128, NPAD), f)
    embT[:, :N_ITEMS] = emb_item.T
    embT = embT.astype(bf16)
    embg = emb_item.astype(bf16)

    lgrid, dgrid = np.meshgrid(np.arange(L), np.arange(D3), indexing="ij")
    BKp = bk[(100 * dgrid + lgrid) % D3].astype(f)      # [L, D3]
    WqT = Wq.T
    Wc = WqT @ BKp.T                                    # [384, 100]
    cq = bq @ BKp.T                                     # [100]
    WkT = Wk.T
    rhsK = np.stack([WkT[128 * j:128 * (j + 1)] for j in range(3)]).astype(bf16)
    rhsB = np.zeros((3, 128, TABB_W), f)
    for j in range(3):
        rhsB[j, :, 0:D3] = WqT[128 * j:128 * (j + 1)]
        rhsB[j, :, D3:D3 + L] = Wc[128 * j:128 * (j + 1)]
        rhsB[j, :, D3 + L] = bv[128 * j:128 * (j + 1)]
    rhsB = rhsB.astype(bf16)
    wv3 = np.stack([Wv[128 * j:128 * (j + 1)] for j in range(3)]).astype(bf16)
    consts = np.zeros((1, D3 + L), f)
    consts[0, :D3] = bq
    consts[0, D3:] = cq
    consts = consts.astype(bf16)

    sq = q.sum(0, dtype=np.float64).astype(f)           # [256]
    sqr = np.zeros((1, D2 + 1), f)
    sqr[0, :D2] = sq
    sqr[0, D2] = float(N_ITEMS)
    sqc = np.ascontiguousarray(sq.reshape(2, 128).T)    # [128, 2]

    shared = dict(qt2=qt2, qe=qe, embT=embT, embg=embg, rhsK=rhsK, rhsB=rhsB,
                  wv3=wv3, consts=consts, sqr=sqr, sqc=sqc)
    in_maps = []
    for c in range(NCORES):
        usr = user[c * BSH:(c + 1) * BSH]
        itm = np.stack([item_i[c * BSH:(c + 1) * BSH],
                        item_j[c * BSH:(c + 1) * BSH]], 1).astype(np.int32)
        m = dict(shared)
        m["user"] = np.ascontiguousarray(usr)
        # dma_gather index plane: logical idx i = l*BSH + b -> u[b, l],
        # laid out [16, n/16] (i%16, i//16), replicated to 8 Q7 stripes
        flat = usr.T.reshape(-1).astype(np.int16)          # i = l*128 + b
        idx16 = np.ascontiguousarray(flat.reshape(-1, 16).T)
        m["userg"] = np.tile(idx16, (8, 1))
        m["item"] = itm
        in_maps.append(m)
    return in_maps


def score_bound(inputs):
    """Rigorous |scores| bound via Cauchy-Schwarz: max||q|| * max||k|| / 16.
    q and k are column-swapped copies of each other, so the norms coincide."""
    emb_in = np.asarray(inputs["emb_in"], dtype=np.float32)
    emb_out = np.asarray(inputs["emb_out"], dtype=np.float32)
    nrm2 = (emb_in * emb_in).sum(1) + (emb_out * emb_out).sum(1)
    return float(nrm2.max()) / np.sqrt(2.0 * D)




def _build_exact(repeat=1, phases="ABCD"):
    import concourse.bass as bass
    import concourse.tile as tile
    from concourse import bacc, mybir
    from concourse.masks import make_identity

    F32 = mybir.dt.float32
    BF = mybir.dt.bfloat16
    I32 = mybir.dt.int32
    MUL = mybir.AluOpType.mult
    ADD = mybir.AluOpType.add

    nc = bacc.Bacc("TRN2", target_bir_lowering=False, debug=False,
                   num_devices=NCORES)

    def din(name, shape, dt):
        return nc.dram_tensor(name, shape, dt, kind="ExternalInput").ap()

    kt_d = din("kt", [2, 128, NPAD], BF)
    qt_d = din("qt", [2, 128, NSH], BF)
    qe_d = din("qe", [NCH, 128, D2 + 1], BF)
    embT_d = din("embT", [128, NPAD], BF)
    embg_d = din("embg", [N_ITEMS, D], BF)
    rhsK_d = din("rhsK", [3, 128, D3], BF)
    rhsB_d = din("rhsB", [3, 128, TABB_W], BF)
    wv3_d = din("wv3", [3, 128, D3], BF)
    consts_d = din("consts", [1, D3 + L], BF)
    user_d = din("user", [BSH, L], I32)
    item_d = din("item", [BSH, 2], I32)
    pred_d = nc.dram_tensor("pred", [BSH, 2], F32, kind="ExternalOutput").ap()

    with tile.TileContext(nc) as tc:
        with (
            tc.tile_pool(name="persist", bufs=1) as pp,
            tc.tile_pool(name="dram", bufs=1, space="DRAM") as dr,
        ):
            reg_sh = dr.tile([NSH, D2], BF)
            reg_full = dr.tile([N_ITEMS, D2], BF)
            tabKG = dr.tile([NPAD, D3 + D2], BF)
            tabB = dr.tile([NPAD, TABB_W], BF)

            # persistent small tiles
            ident = pp.tile([128, 128], BF)
            make_identity(nc, ident[:])
            user_t = pp.tile([BSH, L], I32)
            nc.sync.dma_start(user_t[:], user_d[:])
            item_t = pp.tile([BSH, 2], I32)
            nc.sync.dma_start(item_t[:], item_d[:])
            crow = pp.tile([1, D3 + L], BF)
            nc.sync.dma_start(crow[:], consts_d[:])
            crep = pp.tile([128, D3 + L], BF)
            nc.gpsimd.partition_broadcast(crep[:], crow[:])

            def _phase_a(rep):
                with (
                    tc.tile_pool(name=f"pa{rep}", bufs=1) as pa,
                    tc.tile_pool(name=f"pa_w{rep}", bufs=3) as pw,
                    tc.tile_pool(name=f"pa_ps{rep}", bufs=2, space="PSUM") as pps,
                    tc.tile_pool(name=f"pa_pr{rep}", bufs=1, space="PSUM") as ppr,
                ):
                    kt_sb = pa.tile([128, 2, NPAD], BF)
                    nc.sync.dma_start(kt_sb[:], kt_d[:].rearrange("c p n -> p c n"))
                    qt_sb = pa.tile([128, 2, NSH], BF)
                    nc.sync.dma_start(qt_sb[:], qt_d[:].rearrange("c p m -> p c m"))
                    qe_sb = pa.tile([128, NCH, D2 + 1], BF)
                    nc.sync.dma_start(qe_sb[:], qe_d[:].rearrange("c p w -> p c w"))

                    for m0, mbw in MBLOCKS:
                        nsub = (mbw + 127) // 128
                        psr = [ppr.tile([128, D2 + 1], F32, tag=f"psr{i}",
                                        name=f"psr{i}_{rep}")
                               for i in range(nsub)]
                        for ci in range(NCH):
                            psum_s = pps.tile([128, mbw], F32, tag="psum_s")
                            for kc in range(2):
                                nc.tensor.matmul(
                                    psum_s[:],
                                    kt_sb[:, kc, ci * 128:(ci + 1) * 128],
                                    qt_sb[:, kc, m0:m0 + mbw],
                                    start=(kc == 0), stop=(kc == 1))
                            e_sb = pw.tile([128, mbw], BF, tag="e_sb")
                            nc.scalar.activation(
                                e_sb[:], psum_s[:],
                                mybir.ActivationFunctionType.Exp, scale=1.0 / 16.0)
                            for si in range(nsub):
                                sw = min(128, mbw - si * 128)
                                nc.tensor.matmul(
                                    psr[si][:sw, :],
                                    e_sb[:, si * 128:si * 128 + sw],
                                    qe_sb[:, ci, :],
                                    start=(ci == 0), stop=(ci == NCH - 1))
                        for si in range(nsub):
                            r0 = m0 + si * 128
                            rows = min(128, NSH - r0)
                            rden = pw.tile([128, 1], F32, tag="rden")
                            nc.vector.reciprocal(rden[:rows], psr[si][:rows, D2:D2 + 1])
                            regmb = pw.tile([128, D2], BF, tag="regmb")
                            nc.vector.tensor_scalar_mul(
                                regmb[:rows], psr[si][:rows, 0:D2], rden[:rows])
                            nc.sync.dma_start(reg_sh[r0:r0 + rows, :], regmb[:rows])

            # ---------------- Phase B: AllGather region ----------------
            def _phase_b(rep):
                nc.gpsimd.collective_compute(
                    "AllGather", mybir.AluOpType.bypass,
                    replica_groups=[list(range(NCORES))],
                    ins=[reg_sh.opt()], outs=[reg_full.opt()])

            # ---------------- Phase C: tables ----------------
            def _phase_c(rep):
                with (
                    tc.tile_pool(name=f"pc{rep}", bufs=1) as pc,
                    tc.tile_pool(name=f"pc_w{rep}", bufs=3) as pcw,
                    tc.tile_pool(name=f"pc_ps{rep}", bufs=2, space="PSUM") as pcps,
                ):
                    et_sb = pc.tile([128, NPAD], BF)
                    nc.sync.dma_start(et_sb[:], embT_d[:])
                    rgT = pc.tile([128, 2, NPAD], BF)
                    nc.gpsimd.memset(rgT[:], 0.0)
                    for kc in range(2):
                        nc.sync.dma_start_transpose(
                            rgT[:, kc, 0:N_ITEMS],
                            reg_full[:, kc * 128:(kc + 1) * 128])
                    rk_sb = pc.tile([128, 3, D3], BF)
                    nc.sync.dma_start(rk_sb[:], rhsK_d[:].rearrange("c p w -> p c w"))
                    rb_sb = pc.tile([128, 3, TABB_W], BF)
                    nc.sync.dma_start(rb_sb[:], rhsB_d[:].rearrange("c p w -> p c w"))
                    wv_sb = pc.tile([128, 3, D3], BF)
                    nc.sync.dma_start(wv_sb[:], wv3_d[:].rearrange("c p w -> p c w"))

                    # targets: gather [emb | region] rows for item_i / item_j
                    tgt = pc.tile([128, 2, D3], BF)
                    for s in range(2):
                        nc.gpsimd.indirect_dma_start(
                            out=tgt[:, s, 0:D], out_offset=None, in_=embg_d[:],
                            in_offset=bass.IndirectOffsetOnAxis(
                                ap=item_t[:, s:s + 1], axis=0))
                        nc.gpsimd.indirect_dma_start(
                            out=tgt[:, s, D:D3], out_offset=None, in_=reg_full[:],
                            in_offset=bass.IndirectOffsetOnAxis(
                                ap=item_t[:, s:s + 1], axis=0))
                    # transpose targets -> tcT [feat, (pos128|neg128)]
                    tcT = pc.tile([128, 3, 2 * BSH], BF)
                    for oc in range(3):
                        for s in range(2):
                            pstr = pcps.tile([128, 128], BF, tag="pstr", bufs=1)
                            nc.tensor.transpose(
                                pstr[:], tgt[:, s, oc * 128:(oc + 1) * 128], ident[:])
                            nc.vector.tensor_copy(
                                tcT[:, oc, s * BSH:(s + 1) * BSH], pstr[:])
                    # M[in, tgt] = sum_out Wv[out, in] * tcT[out, tgt]
                    grhs = pc.tile([128, 3, 2 * BSH], BF)
                    for ic in range(3):
                        psM = pcps.tile([128, 2 * BSH], F32, tag="psM", bufs=1)
                        for oc in range(3):
                            nc.tensor.matmul(
                                psM[:], wv_sb[:, oc, ic * 128:(ic + 1) * 128],
                                tcT[:, oc, :], start=(oc == 0), stop=(oc == 2))
                        nc.vector.tensor_copy(grhs[:, ic, :], psM[:])

                    # table matmuls, 79 chunks of 128 items
                    for ch in range(NCH):
                        sl = slice(ch * 128, (ch + 1) * 128)
                        psK = pcps.tile([128, D3], F32, tag="psK")
                        psG = pcps.tile([128, D2], F32, tag="psG")
                        psB = pcps.tile([128, TABB_W], F32, tag="psB")
                        for j in range(3):
                            lh = et_sb[:, sl] if j == 0 else rgT[:, j - 1, sl]
                            nc.tensor.matmul(psK[:], lh, rk_sb[:, j, :],
                                             start=(j == 0), stop=(j == 2))
                            nc.tensor.matmul(psG[:], lh, grhs[:, j, :],
                                             start=(j == 0), stop=(j == 2))
                            nc.tensor.matmul(psB[:], lh, rb_sb[:, j, :],
                                             start=(j == 0), stop=(j == 2))
                        cKG = pcw.tile([128, D3 + D2], BF, tag="cKG")
                        nc.vector.tensor_copy(cKG[:, 0:D3], psK[:])
                        nc.scalar.copy(cKG[:, D3:D3 + D2], psG[:])
                        nc.sync.dma_start(tabKG[sl, :], cKG[:])
                        cB = pcw.tile([128, TABB_W], BF, tag="cB")
                        nc.vector.tensor_copy(cB[:], psB[:])
                        nc.sync.dma_start(tabB[sl, :], cB[:])

            # ---------------- Phase D: attention_network ----------------
            def _phase_d(rep):
                with (
                    tc.tile_pool(name=f"pd{rep}", bufs=1) as pd,
                    tc.tile_pool(name=f"pd_w{rep}", bufs=2) as pdw,
                ):
                    # gathers
                    ke = pd.tile([128, L, D3], BF)
                    bi = pd.tile([128, 2, TABB_W], BF)
                    for s in range(2):
                        nc.gpsimd.indirect_dma_start(
                            out=bi[:, s, :], out_offset=None, in_=tabB[:],
                            in_offset=bass.IndirectOffsetOnAxis(
                                ap=item_t[:, s:s + 1], axis=0))

                    preds = pd.tile([128, 2], F32)
                    ke_scr = ke[:].rearrange("p a b -> p (a b)").rearrange(
                        "p (d l) -> p l d", l=L)  # [128, L, D3] scrambled view

                    dens_all = pd.tile([128, 2], F32)
                    num_all = pd.tile([128, 2], F32)
                    bvd_all = pd.tile([128, 2], F32)

                    # merged K|G gather, l-chunked; diag-extract G in place
                    u0i = pd.tile([128, L, 2], F32)
                    LCH = 25
                    for l0 in range(0, L, LCH):
                        kg = pdw.tile([128, LCH, D3 + D2], BF, tag="kg", bufs=1)
                        for l in range(l0, l0 + LCH):
                            nc.gpsimd.indirect_dma_start(
                                out=kg[:, l - l0, :], out_offset=None,
                                in_=tabKG[:],
                                in_offset=bass.IndirectOffsetOnAxis(
                                    ap=user_t[:, l:l + 1], axis=0))
                        nc.scalar.copy(ke[:, l0:l0 + LCH, :], kg[:, :, 0:D3])
                        prod = pdw.tile([128, LCH, 2, BSH], BF, tag="prod")
                        nc.vector.tensor_tensor(
                            out=prod[:],
                            in0=kg[:, :, D3:D3 + D2].rearrange(
                                "p a (s t) -> p a s t", s=2),
                            in1=ident[:].unsqueeze(1).unsqueeze(1).to_broadcast(
                                [128, LCH, 2, BSH]),
                            op=MUL)
                        nc.vector.tensor_reduce(
                            u0i[:, l0:l0 + LCH, :].rearrange("p a b -> p (a b)"),
                            prod[:], axis=mybir.AxisListType.X, op=ADD)

                    for s in range(2):
                        qp = pdw.tile([128, D3], BF, tag="qp")
                        nc.vector.tensor_tensor(out=qp[:], in0=bi[:, s, 0:D3],
                                                in1=crep[:, 0:D3], op=ADD)
                        ct = pdw.tile([128, L], F32, tag="ct")
                        nc.vector.tensor_tensor(out=ct[:], in0=bi[:, s, D3:D3 + L],
                                                in1=crep[:, D3:D3 + L], op=ADD)
                        if s == 0:
                            eq = pdw.tile([128, L], F32, tag="eq")
                            nc.vector.tensor_tensor(
                                out=eq[:], in0=user_t[:],
                                in1=item_t[:, 0:1].to_broadcast([BSH, L]),
                                op=mybir.AluOpType.is_equal)
                            pen = pdw.tile([128, L], F32, tag="pen")
                            nc.vector.tensor_scalar_mul(pen[:], eq[:], PEN)
                            nc.vector.tensor_tensor(out=ct[:], in0=ct[:], in1=pen[:],
                                                    op=ADD)
                        # s0 via scrambled dot, l-chunked
                        s0 = pdw.tile([128, L], F32, tag="s0")
                        for l0 in range(0, L, LCH):
                            z = pdw.tile([128, LCH, D3], BF, tag="z")
                            nc.vector.tensor_tensor(
                                out=z[:], in0=ke_scr[:, l0:l0 + LCH, :],
                                in1=qp[:].unsqueeze(1).to_broadcast([128, LCH, D3]),
                                op=MUL)
                            nc.vector.tensor_reduce(
                                s0[:, l0:l0 + LCH], z[:],
                                axis=mybir.AxisListType.X, op=ADD)
                        nc.vector.tensor_tensor(out=s0[:], in0=s0[:], in1=ct[:], op=ADD)
                        expa = pdw.tile([128, L], F32, tag="expa")
                        den = pdw.tile([128, 1], F32, tag="den")
                        nc.scalar.activation(
                            expa[:], s0[:], mybir.ActivationFunctionType.Exp,
                            scale=float(1.0 / np.sqrt(D3)), accum_out=den[:])
                        nc.scalar.sqrt(dens_all[:, s:s + 1], den[:])
                        # num = sum_l expa * u0
                        wu = pdw.tile([128, L], F32, tag="wu")
                        nc.vector.tensor_tensor(out=wu[:], in0=expa[:],
                                                in1=u0i[:, :, s], op=MUL)
                        nc.vector.tensor_reduce(num_all[:, s:s + 1], wu[:],
                                                axis=mybir.AxisListType.X, op=ADD)
                        nc.vector.tensor_copy(bvd_all[:, s:s + 1],
                                              bi[:, s, D3 + L:D3 + L + 1])

                    # pred = num / dens + bvdot * dens
                    rdens = pd.tile([128, 2], F32)
                    nc.vector.reciprocal(rdens[:], dens_all[:])
                    t1 = pd.tile([128, 2], F32)
                    nc.vector.tensor_tensor(out=t1[:], in0=num_all[:], in1=rdens[:],
                                            op=MUL)
                    t2 = pd.tile([128, 2], F32)
                    nc.vector.tensor_tensor(out=t2[:], in0=bvd_all[:],
                                            in1=dens_all[:], op=MUL)
                    nc.vector.tensor_tensor(out=preds[:], in0=t1[:], in1=t2[:], op=ADD)
                    nc.sync.dma_start(pred_d[:], preds[:])

            for rep in range(repeat):
                if "A" in phases:
                    _phase_a(rep)
                if "B" in phases:
                    _phase_b(rep)
                if "C" in phases:
                    _phase_c(rep)
                if "D" in phases:
                    _phase_d(rep)

    nc.compile()
    return nc


def _prep_exact(inputs):
    f = np.float32
    user = np.asarray(inputs["user"]).astype(np.int32)
    item_i = np.asarray(inputs["item_i"]).astype(np.int32)
    item_j = np.asarray(inputs["item_j"]).astype(np.int32)
    emb_item = np.asarray(inputs["emb_item"], dtype=f)
    emb_in = np.asarray(inputs["emb_in"], dtype=f)
    emb_out = np.asarray(inputs["emb_out"], dtype=f)
    Wq = np.asarray(inputs["Wq"], dtype=f)
    bq = np.asarray(inputs["bq"], dtype=f)
    Wk = np.asarray(inputs["Wk"], dtype=f)
    bk = np.asarray(inputs["bk"], dtype=f)
    Wv = np.asarray(inputs["Wv"], dtype=f)
    bv = np.asarray(inputs["bv"], dtype=f)

    q = np.concatenate([emb_in, emb_out], 1)            # [N, 256]
    k = np.concatenate([emb_out, emb_in], 1)
    kT = np.zeros((D2, NPAD), f)
    kT[:, :N_ITEMS] = k.T
    kt = kT.reshape(2, 128, NPAD).astype(bf16)
    qT = np.ascontiguousarray(q.T)                      # [256, 10000]
    qe = np.zeros((NPAD, D2 + 1), f)
    qe[:N_ITEMS, :D2] = q
    qe[:N_ITEMS, D2] = 1.0
    qe = qe.reshape(NCH, 128, D2 + 1).astype(bf16)
    embT = np.zeros((128, NPAD), f)
    embT[:, :N_ITEMS] = emb_item.T
    embT = embT.astype(bf16)
    embg = emb_item.astype(bf16)

    lgrid, dgrid = np.meshgrid(np.arange(L), np.arange(D3), indexing="ij")
    BKp = bk[(100 * dgrid + lgrid) % D3].astype(f)      # [L, D3]
    WqT = Wq.T
    Wc = WqT @ BKp.T                                    # [384, 100]
    cq = bq @ BKp.T                                     # [100]
    WkT = Wk.T
    rhsK = np.stack([WkT[128 * j:128 * (j + 1)] for j in range(3)]).astype(bf16)
    rhsB = np.zeros((3, 128, TABB_W), f)
    for j in range(3):
        rhsB[j, :, 0:D3] = WqT[128 * j:128 * (j + 1)]
        rhsB[j, :, D3:D3 + L] = Wc[128 * j:128 * (j + 1)]
        rhsB[j, :, D3 + L] = bv[128 * j:128 * (j + 1)]
    rhsB = rhsB.astype(bf16)
    wv3 = np.stack([Wv[128 * j:128 * (j + 1)] for j in range(3)]).astype(bf16)
    consts = np.zeros((1, D3 + L), f)
    consts[0, :D3] = bq
    consts[0, D3:] = cq
    consts = consts.astype(bf16)

    shared = dict(kt=kt, qe=qe, embT=embT, embg=embg, rhsK=rhsK, rhsB=rhsB,
                  wv3=wv3, consts=consts)
    in_maps = []
    for c in range(NCORES):
        qts = np.ascontiguousarray(
            qT[:, c * NSH:(c + 1) * NSH]).reshape(2, 128, NSH).astype(bf16)
        usr = user[c * BSH:(c + 1) * BSH]
        itm = np.stack([item_i[c * BSH:(c + 1) * BSH],
                        item_j[c * BSH:(c + 1) * BSH]], 1).astype(np.int32)
        m = dict(shared)
        m["qt"] = qts
        m["user"] = np.ascontiguousarray(usr)
        m["item"] = itm
        in_maps.append(m)
    return in_maps




_CACHE = {}

# aliases used by test.py / bench tooling (linear path is the default)
def _build_program(repeat=1, phases="ACD"):
    return _build_linear(repeat=repeat, phases=phases)


def _prep_inputs(inputs):
    return _prep_linear(inputs)


def kernel(**inputs):
    from concourse.bass_utils import run_bass_kernel_spmd
    use_linear = score_bound(inputs) <= SCORE_BOUND
    key = "nc_lin" if use_linear else "nc_exact"
    if key not in _CACHE:
        _CACHE[key] = _build_linear() if use_linear else _build_exact()
    nc = _CACHE[key]
    in_maps = _prep_linear(inputs) if use_linear else _prep_exact(inputs)
    res = run_bass_kernel_spmd(nc, in_maps, list(range(NCORES))).results
    out = np.concatenate([res[c]["pred"] for c in range(NCORES)])
    if use_linear:  # device left [num + bvd*S | S]; finish pred here
        out = out[:, 0:2] / np.sqrt(out[:, 2:4])
    return (np.ascontiguousarray(out[:, 0], dtype=np.float32),
            np.ascontiguousarray(out[:, 1], dtype=np.float32))



# revision 28
# speedup vs baseline: 1.0401x; 1.0401x over previous
"""Trainium2 Bass kernel for nn_New3_77395310674432 (sparse_attention).

Pipeline (8-core SPMD, one NEFF, NO collectives), default "linear" path:
  A') region via linearized softmax: scores s = q@k.T/16 are provably tiny
      (|s| <= max||q||^2/16 ~ 2.3e-3 for these inputs; the Cauchy-Schwarz
      bound is checked on the host and the exact kernel below is used
      instead if it exceeds SCORE_BOUND). exp(s) ~= 1+s  =>
      region = (sq + q C/16) / (N + q sk/16),  C = K^T Q  [256,256],
      by matmul associativity (QK^T)Q = Q(K^TQ). Computed directly in the
      transposed layout rgT on every core (replicated, ~2.6 GFLOP total)
      -- no N^2 work, no AllGather, no DMA transpose.
  A'+C interleaved: per-core full item tables (factorized projections)
      tabK = feats@Wk.T (384) | tabG = feats@M (256; M = Wv.T@tgt.T for
      this core's 256 targets), written merged per 128-item chunk right
      after its region block; feats = [emb_item | region]. Target B-rows
      (Wq/Wc/bv projections, incl. the reshape-quirk key-bias fold) are
      computed directly from the transposed targets -- no 10k-row B table.
  D) Per-core batch shard (128/core): per-l indirect gathers of merged
      K|G rows by user indices (HW honors one index per partition per
      call), s0 accumulated per gather chunk via the reshape-quirk d-block
      decomposition (flat positions d*100+l of 96 d's == 25 table rows),
      u0 via diag-extract of G, exp/mask/pow(beta=.5) -> predictions.

Perf notes (2026-08-11 session, HW NTFF traces via antenv.axon_hooks shim):
  - s0 multiply reads kec j-flat CONTIGUOUS (d-major [96,100] view); the
    old scrambled [l,d] view ran 2.2 ns/elem on HW vs ~1.05 contiguous.
  - s0/u0 reduces: bf16 fold halves (DVE 2x mode needs every operand's
    LAST ap dim packed >=2 and 2-byte) then one small strided f32 reduce.
  - 1/x via reciprocal_approx_fast (InstReciprocal was 7.8 ns/elem on HW).
  - Copies deliberately on Scalar engine (Act): they pipeline against
    DVE across gather chunks; moving them to DVE regressed (-19%).
  - InstTensorTensorReduce and gpsimd.dma_gather (InstDMAGatherAnt) both
    FAIL on this axon/PJRT runtime (INTERNAL at execute) -- do not use.
    The 100 indirect gathers cost ~1.1us each on GpSimd (descriptor gen),
    ~110us/core total; dma_gather would have cut that to ~8us.
  - Single-shot NTFF exec times vary +-19% with clock/pstate; only the
    repeat-slope (R=1 vs 16) numbers are comparable across builds.

Exact fallback (_build_exact): the original full-softmax kernel (row-
sharded N^2 attention + AllGather), used only if the score bound fails.
"""
import sys
if "/opt/trn_rl_repo" not in sys.path:
    sys.path.insert(0, "/opt/trn_rl_repo")
import numpy as np
import ml_dtypes

bf16 = ml_dtypes.bfloat16

N_ITEMS = 10000
D = 128
D2 = 256
D3 = 384
B = 1024
L = 100
NCORES = 8
BSH = B // NCORES            # 128 batches per core
NPAD = 79 * 128              # 10112 padded items
NCH = 79                     # 128-row chunks
TABB_W = 512                 # [Q0 384 | Crow 100 | bvdot 1 | pad 27]
PEN = -1.0e9
NFCH = 512                   # free-dim chunk for the region matmuls
SCORE_BOUND = 0.05           # |s| above this -> exact fallback

NSH = N_ITEMS // NCORES     # exact-path row shard per core
MBLOCKS = [(0, 512), (512, 512), (1024, 226)]  # exact-path stage-A m-blocks
USE_DMA_GATHER = False      # InstDMAGatherAnt fails on this runtime (HW)




def _build_linear(repeat=1, phases="ACD"):
    import concourse.bass as bass
    import concourse.tile as tile
    from concourse import bacc, mybir
    from concourse.masks import make_identity

    F32 = mybir.dt.float32
    BF = mybir.dt.bfloat16
    I32 = mybir.dt.int32
    MUL = mybir.AluOpType.mult
    ADD = mybir.AluOpType.add

    nc = bacc.Bacc("TRN2", target_bir_lowering=False, debug=False,
                   num_devices=NCORES)

    def din(name, shape, dt):
        return nc.dram_tensor(name, shape, dt, kind="ExternalInput").ap()

    qt2_d = din("qt2", [2, 128, NPAD], BF)          # q^T, zero-padded
    qe_d = din("qe", [NCH, 128, D2 + 1], BF)        # [q | 1] rows, zero-padded
    embT_d = din("embT", [128, NPAD], BF)
    embg_d = din("embg", [N_ITEMS, D], BF)
    rhsK_d = din("rhsK", [3, 128, D3], BF)
    rhsB_d = din("rhsB", [3, 128, TABB_W], BF)
    wv3_d = din("wv3", [3, 128, D3], BF)
    consts_d = din("consts", [1, D3 + L], BF)
    sqr_d = din("sqr", [1, D2 + 1], F32)            # [sum_n q | N]
    sqc_d = din("sqc", [128, 2], F32)               # sq as column per j-half
    user_d = din("user", [BSH, L], I32)
    userg_d = din("userg", [128, L * BSH // 16], mybir.dt.int16)
    item_d = din("item", [BSH, 2], I32)
    pred_d = nc.dram_tensor("pred", [BSH, 4], F32, kind="ExternalOutput").ap()

    nch_sl = []
    n0 = 0
    while n0 < NPAD:
        w = min(NFCH, NPAD - n0)
        nch_sl.append((n0, w))
        n0 += w

    with tile.TileContext(nc) as tc:
        with (
            tc.tile_pool(name="persist", bufs=1) as pp,
            tc.tile_pool(name="dram", bufs=1, space="DRAM") as dr,
        ):
            tabKG = dr.tile([NPAD, D3 + D2], BF)

            ident = pp.tile([128, 128], BF)
            make_identity(nc, ident[:])
            user_t = pp.tile([BSH, L], I32)
            nc.sync.dma_start(user_t[:], user_d[:])
            userg_t = pp.tile([128, L * BSH // 16], mybir.dt.int16)
            nc.sync.dma_start(userg_t[:], userg_d[:])
            item_t = pp.tile([BSH, 2], I32)
            nc.sync.dma_start(item_t[:], item_d[:])
            crow = pp.tile([1, D3 + L], BF)
            nc.sync.dma_start(crow[:], consts_d[:])
            crep = pp.tile([128, D3 + L], BF)
            nc.gpsimd.partition_broadcast(crep[:], crow[:])
            ident2 = pp.tile([128, 2 * BSH], BF)
            nc.vector.tensor_copy(ident2[:, 0:BSH], ident[:])
            nc.vector.tensor_copy(ident2[:, BSH:2 * BSH], ident[:])

            # ---------------- Phase A': linearized region ----------------
            def _phase_a(rep, par):
                rgT = par["rgT"]
                regtg = par["regtg"]
                bi = par["bi"]
                with (
                    tc.tile_pool(name=f"pa{rep}", bufs=1) as pa,
                    tc.tile_pool(name=f"pa_w{rep}", bufs=3) as pw,
                    tc.tile_pool(name=f"pa_cw{rep}", bufs=3) as pcw,
                    tc.tile_pool(name=f"pa_pr{rep}", bufs=1, space="PSUM") as ppr,
                ):
                    # qe first, quartered: C16 accumulation starts on the
                    # first quarter while the rest (and qt2/embT) stream in.
                    qe_sb = pa.tile([128, NCH, D2 + 1], BF)
                    qcuts = [0, 20, 40, 60, NCH]
                    for qi in range(len(qcuts) - 1):
                        nc.sync.dma_start(
                            qe_sb[:, qcuts[qi]:qcuts[qi + 1], :],
                            qe_d[qcuts[qi]:qcuts[qi + 1]].rearrange(
                                "c p w -> p c w"))
                    qt2_sb = pa.tile([128, 2, NPAD], BF)
                    nc.sync.dma_start(qt2_sb[:], qt2_d[:].rearrange("c p n -> p c n"))
                    sqrow = pa.tile([1, D2 + 1], F32)
                    nc.sync.dma_start(sqrow[:], sqr_d[:])
                    sqrep = pa.tile([128, D2 + 1], F32)
                    nc.gpsimd.partition_broadcast(sqrep[:], sqrow[:])
                    sqbf = pa.tile([1, D2], BF)
                    nc.vector.tensor_copy(sqbf[:], sqrow[:, 0:D2])
                    ones_row = pa.tile([1, NFCH], BF)
                    nc.gpsimd.memset(ones_row[:], 1.0)
                    et_sb = pa.tile([128, NPAD], BF)
                    nc.sync.dma_start(et_sb[:], embT_d[:])
                    rk_sb = pa.tile([128, 3, D3], BF)
                    nc.sync.dma_start(rk_sb[:], rhsK_d[:].rearrange("c p w -> p c w"))
                    rb_sb = pa.tile([128, 3, TABB_W], BF)
                    nc.sync.dma_start(rb_sb[:], rhsB_d[:].rearrange("c p w -> p c w"))
                    wv_sb = pa.tile([128, 3, D3], BF)
                    nc.sync.dma_start(wv_sb[:], wv3_d[:].rearrange("c p w -> p c w"))

                    # C_ext[i, j] = sum_n k[n,i] q[n,j]  (+ sk in col 256)
                    psC = [ppr.tile([128, D2 + 1], F32, tag=f"psC{i}",
                                    name=f"psC{i}_{rep}") for i in range(2)]
                    for ci in range(NCH):
                        st, sp = (ci == 0), (ci == NCH - 1)
                        nc.tensor.matmul(psC[0][:], qe_sb[:, ci, 128:256],
                                         qe_sb[:, ci, :], start=st, stop=sp)
                        nc.tensor.matmul(psC[1][:], qe_sb[:, ci, 0:128],
                                         qe_sb[:, ci, :], start=st, stop=sp)
                    C16 = pa.tile([128, 2, D2 + 1], BF)
                    nc.scalar.mul(C16[:, 0, :], psC[0][:], 1.0 / 16.0)
                    nc.scalar.mul(C16[:, 1, :], psC[1][:], 1.0 / 16.0)

                    # per-target region rows + tcT/grhs/bi (psum pool scoped)
                    tgt = pa.tile([128, 2, D3], BF)
                    tcT = pa.tile([128, 3, 2 * BSH], BF)
                    grhs = pa.tile([128, 3, 2 * BSH], BF)
                    with tc.tile_pool(name=f"pa_pt{rep}", bufs=1,
                                      space="PSUM") as ppst:
                        for s in range(2):
                            qg = pa.tile([128, D2 + 1], BF, tag=f"qg{s}")
                            nc.gpsimd.indirect_dma_start(
                                out=qg[:], out_offset=None,
                                in_=qe_d[:].rearrange("c p w -> (c p) w"),
                                in_offset=bass.IndirectOffsetOnAxis(
                                    ap=item_t[:, s:s + 1], axis=0))
                            qgT = pa.tile([128, 2, 128], BF, tag=f"qgT{s}")
                            for ic in range(2):
                                pstr = ppst.tile([128, 128], BF, tag="pstr_sq")
                                nc.tensor.transpose(
                                    pstr[:], qg[:, ic * 128:(ic + 1) * 128],
                                    ident[:])
                                nc.scalar.copy(qgT[:, ic, :], pstr[:])
                            psT = ppst.tile([128, D2 + 1], F32, tag="psT")
                            nc.tensor.matmul(psT[:], qgT[:, 0, :], C16[:, 0, :],
                                             start=True, stop=False)
                            nc.tensor.matmul(psT[:], qgT[:, 1, :], C16[:, 1, :],
                                             start=False, stop=True)
                            tgta = pw.tile([128, D2 + 1], F32, tag="tgta")
                            nc.vector.tensor_tensor(out=tgta[:], in0=psT[:],
                                                    in1=sqrep[:], op=ADD)
                            rdent = pw.tile([128, 1], F32, tag="rdent")
                            nc.vector.reciprocal_approx_fast(
                                rdent[:], tgta[:, D2:D2 + 1])
                            nc.vector.tensor_scalar_mul(
                                regtg[:, s, :], tgta[:, 0:D2], rdent[:])
                            # targets: [emb | region] rows
                            nc.gpsimd.indirect_dma_start(
                                out=tgt[:, s, 0:D], out_offset=None,
                                in_=embg_d[:],
                                in_offset=bass.IndirectOffsetOnAxis(
                                    ap=item_t[:, s:s + 1], axis=0))
                            nc.scalar.copy(tgt[:, s, D:D3], regtg[:, s, :])
                        # transpose targets -> tcT [feat, (pos128|neg128)]
                        for oc in range(3):
                            for s in range(2):
                                pstr = ppst.tile([128, 128], BF, tag="pstr_sq")
                                nc.tensor.transpose(
                                    pstr[:], tgt[:, s, oc * 128:(oc + 1) * 128],
                                    ident[:])
                                nc.scalar.copy(
                                    tcT[:, oc, s * BSH:(s + 1) * BSH], pstr[:])
                        # M[in, tgt] = sum_out Wv[out, in] * tcT[out, tgt]
                        for ic in range(3):
                            psM = ppst.tile([128, 2 * BSH], F32, tag="psT")
                            for oc in range(3):
                                nc.tensor.matmul(
                                    psM[:], wv_sb[:, oc, ic * 128:(ic + 1) * 128],
                                    tcT[:, oc, :], start=(oc == 0),
                                    stop=(oc == 2))
                            nc.vector.tensor_copy(grhs[:, ic, :], psM[:])
                        # target B-rows directly from tcT (no 10k-item table)
                        for s in range(2):
                            psTB = ppst.tile([128, TABB_W], F32, tag="psTB")
                            for oc in range(3):
                                nc.tensor.matmul(
                                    psTB[:], tcT[:, oc, s * BSH:(s + 1) * BSH],
                                    rb_sb[:, oc, :], start=(oc == 0),
                                    stop=(oc == 2))
                            nc.scalar.copy(bi[:, s, :], psTB[:])

                    # interleaved: region chunk n, then table chunks 4n..4n+3
                    reg_ps = tc.tile_pool(name=f"pa_rg{rep}", bufs=1,
                                          space="PSUM")
                    ppsr = reg_ps.__enter__()
                    for nci, (n0, w) in enumerate(nch_sl):
                        sl = slice(n0, n0 + w)
                        psD = ppsr.tile([1, NFCH], F32, tag="psD", bufs=1)
                        nc.tensor.matmul(psD[:, :w], C16[:, 0, 256:257],
                                         qt2_sb[:, 0, sl], start=True, stop=False)
                        nc.tensor.matmul(psD[:, :w], C16[:, 1, 256:257],
                                         qt2_sb[:, 1, sl], start=False, stop=True)
                        drec = pw.tile([1, NFCH], F32, tag="drec")
                        nc.vector.tensor_scalar_add(drec[:, :w], psD[:, :w],
                                                    float(N_ITEMS))
                        nc.vector.reciprocal_approx_fast(drec[:, :w],
                                                         drec[:, :w])
                        drecb = pw.tile([128, NFCH], F32, tag="drecb")
                        nc.gpsimd.partition_broadcast(drecb[:, :w],
                                                      drec[:, :w])
                        psNT2 = ppsr.tile([128, 2, NFCH], F32, tag="psNT",
                                          bufs=1)
                        for jh in range(2):
                            nc.tensor.matmul(
                                psNT2[:, jh, :w],
                                C16[:, 0, jh * 128:(jh + 1) * 128],
                                qt2_sb[:, 0, sl], start=True, stop=False)
                            nc.tensor.matmul(
                                psNT2[:, jh, :w],
                                C16[:, 1, jh * 128:(jh + 1) * 128],
                                qt2_sb[:, 1, sl], start=False, stop=False)
                            nc.tensor.matmul(
                                psNT2[:, jh, :w],
                                sqbf[0:1, jh * 128:(jh + 1) * 128],
                                ones_row[0:1, :w], start=False, stop=True)
                        nc.vector.tensor_tensor(
                            out=rgT[:, :, sl], in0=psNT2[:, :, :w],
                            in1=drecb[:, :w].unsqueeze(1).to_broadcast(
                                [128, 2, w]),
                            op=MUL)
                        # table chunks covered by this region block
                        for ch in range(4 * nci, min(4 * nci + 4, NCH)):
                            tsl = slice(ch * 128, (ch + 1) * 128)
                            psK = ppsr.tile([128, D3], F32, tag="psK")
                            psG = ppsr.tile([128, D2], F32, tag="psG")
                            for j in range(3):
                                lh = (et_sb[:, tsl] if j == 0
                                      else rgT[:, j - 1, tsl])
                                nc.tensor.matmul(psK[:], lh, rk_sb[:, j, :],
                                                 start=(j == 0), stop=(j == 2))
                                nc.tensor.matmul(psG[:], lh, grhs[:, j, :],
                                                 start=(j == 0), stop=(j == 2))
                            cKG = pcw.tile([128, D3 + D2], BF, tag="cKG")
                            nc.scalar.copy(cKG[:, 0:D3], psK[:])
                            nc.scalar.copy(cKG[:, D3:D3 + D2], psG[:])
                            nc.sync.dma_start(tabKG[tsl, :], cKG[:])
                    reg_ps.__exit__(None, None, None)

            def _phase_c(rep, par):
                pass

            # ---------------- Phase G: gathers only (bench probe) ------
            def _phase_g(rep, par):
                with (
                    tc.tile_pool(name=f"pg{rep}", bufs=1) as pg,
                    tc.tile_pool(name=f"pg_w{rep}", bufs=2) as pgw,
                ):
                    preds = pg.tile([128, 2], F32)
                    LCH = 25
                    for dc in range(L // LCH):
                        l0 = dc * LCH
                        kg = pgw.tile([128, LCH, D3 + D2], BF, tag="kg")
                        for l in range(l0, l0 + LCH):
                            nc.gpsimd.indirect_dma_start(
                                out=kg[:, l - l0, :], out_offset=None,
                                in_=tabKG[:],
                                in_offset=bass.IndirectOffsetOnAxis(
                                    ap=user_t[:, l:l + 1], axis=0))
                        nc.vector.tensor_copy(preds[:], kg[:, 0, 0:2])
                    nc.sync.dma_start(pred_d[:], preds[:])

            # ---------------- Phase D: attention_network ----------------
            def _phase_d(rep, par):
                bi = par["bi"]
                with (
                    tc.tile_pool(name=f"pd{rep}", bufs=1) as pd,
                    tc.tile_pool(name=f"pd_w{rep}", bufs=2) as pdw,
                ):
                    denb = pd.tile([128, 2], F32)
                    num_all = pd.tile([128, 2], F32)
                    bvd_all = pd.tile([128, 2], F32)

                    # qp first: s0 partials need it inside the gather loop
                    qp2 = pd.tile([128, 2, D3], BF)
                    for s in range(2):
                        nc.vector.tensor_tensor(out=qp2[:, s, :],
                                                in0=bi[:, s, 0:D3],
                                                in1=crep[:, 0:D3], op=ADD)

                    # merged K|G gather, l-chunked; per chunk: diag-extract G
                    # and accumulate the s0 d-chunk partial (the reshape quirk
                    # maps flat [d*100+l for 96 d's] onto exactly 25 ke rows).
                    u0i = pd.tile([128, L, 2], F32)
                    s0b = pd.tile([128, 2, L], F32)
                    LCH = 25
                    DCH = 96
                    NIDX = LCH * 128
                    for dc in range(L // LCH):
                        l0 = dc * LCH
                        kg = pdw.tile([128, LCH, D3 + D2], BF, tag="kg")
                        if USE_DMA_GATHER:
                            # one batched gather: row u[b,l0+c] -> kg[b,c,:]
                            # (idx i=c*128+b at userg[i%16, dc*200 + i//16])
                            nc.gpsimd.dma_gather(
                                kg[:], tabKG[:],
                                userg_t[:, dc * (NIDX // 16):
                                        (dc + 1) * (NIDX // 16)],
                                NIDX, NIDX, D3 + D2)
                        else:
                            for l in range(l0, l0 + LCH):
                                nc.gpsimd.indirect_dma_start(
                                    out=kg[:, l - l0, :], out_offset=None,
                                    in_=tabKG[:],
                                    in_offset=bass.IndirectOffsetOnAxis(
                                        ap=user_t[:, l:l + 1], axis=0))
                        kec = pdw.tile([128, LCH, D3], BF, tag="kec", bufs=2)
                        nc.scalar.copy(kec[:], kg[:, :, 0:D3])
                        prod = pdw.tile([128, LCH, D2], BF, tag="prod", bufs=1)
                        nc.vector.tensor_tensor(
                            out=prod[:],
                            in0=kg[:, :, D3:D3 + D2],
                            in1=ident2[:].unsqueeze(1).to_broadcast(
                                [128, LCH, D2]),
                            op=MUL)
                        # u0: fold 128-wide diag segments 128->16 in bf16
                        # (2x DVE mode: last AP dim packed), then f32 reduce.
                        pv = prod[:].rearrange("p a (s t) -> p a s t", s=2)
                        uf1 = pdw.tile([128, LCH, 2, 64], BF, tag="uf1", bufs=1)
                        nc.vector.tensor_tensor(
                            out=uf1[:], in0=pv[:, :, :, 0:64],
                            in1=pv[:, :, :, 64:128], op=ADD)
                        uf2 = pdw.tile([128, LCH, 2, 32], BF, tag="uf2", bufs=1)
                        nc.vector.tensor_tensor(
                            out=uf2[:], in0=uf1[:, :, :, 0:32],
                            in1=uf1[:, :, :, 32:64], op=ADD)
                        uf3 = pdw.tile([128, LCH, 2, 16], BF, tag="uf3", bufs=1)
                        nc.vector.tensor_tensor(
                            out=uf3[:], in0=uf2[:, :, :, 0:16],
                            in1=uf2[:, :, :, 16:32], op=ADD)
                        nc.vector.tensor_reduce(
                            u0i[:, l0:l0 + LCH, :].rearrange("p a b -> p (a b)"),
                            uf3[:].rearrange("p a s t -> p (a s) t"),
                            axis=mybir.AxisListType.X, op=ADD)
                        # zc2 in d-major layout, in two 48-d halves (halves
                        # the SBUF working set so kec can double-buffer):
                        # in0 = kec j-flat (contiguous, packed innermost) so
                        # the multiply avoids the HW strided-read penalty.
                        kec_dm = kec[:].rearrange("p a b -> p (a b)").rearrange(
                            "p (d l) -> p d l", l=L)  # [128, 96, 100] contig
                        DH = DCH // 2
                        for dh in range(2):
                            d0 = dc * DCH + dh * DH
                            zc2 = pdw.tile([128, 2, DH, L], BF, tag="zc",
                                           bufs=1)
                            nc.vector.tensor_tensor(
                                out=zc2[:],
                                in0=kec_dm[:, dh * DH:(dh + 1) * DH, :]
                                    .unsqueeze(1).to_broadcast([128, 2, DH, L]),
                                in1=qp2[:, :, d0:d0 + DH]
                                    .unsqueeze(3).to_broadcast([128, 2, DH, L]),
                                op=MUL)
                            # fold d 48 -> 6 in bf16 (2x mode, contiguous
                            # halves), then strided f32 reduce over d
                            zf1 = pdw.tile([128, 2, DH // 2, L], BF,
                                           tag="zf1", bufs=1)
                            nc.vector.tensor_tensor(
                                out=zf1[:], in0=zc2[:, :, 0:24, :],
                                in1=zc2[:, :, 24:48, :], op=ADD)
                            zf2 = pdw.tile([128, 2, DH // 4, L], BF,
                                           tag="zf2", bufs=1)
                            nc.vector.tensor_tensor(
                                out=zf2[:], in0=zf1[:, :, 0:12, :],
                                in1=zf1[:, :, 12:24, :], op=ADD)
                            zf3 = pdw.tile([128, 2, DH // 8, L], BF,
                                           tag="zf3", bufs=1)
                            nc.vector.tensor_tensor(
                                out=zf3[:], in0=zf2[:, :, 0:6, :],
                                in1=zf2[:, :, 6:12, :], op=ADD)
                            zf3v = zf3[:].rearrange("p s d l -> p s l d")
                            if dc == 0 and dh == 0:
                                nc.vector.tensor_reduce(
                                    s0b[:].rearrange("p a b -> p (a b)"), zf3v,
                                    axis=mybir.AxisListType.X, op=ADD)
                            else:
                                part = pdw.tile([128, 2, L], F32, tag="part")
                                nc.vector.tensor_reduce(
                                    part[:].rearrange("p a b -> p (a b)"),
                                    zf3v, axis=mybir.AxisListType.X, op=ADD)
                                nc.vector.tensor_tensor(
                                    out=s0b[:], in0=s0b[:], in1=part[:],
                                    op=ADD)

                    for s in range(2):
                        ct = pdw.tile([128, L], F32, tag="ct")
                        nc.vector.tensor_tensor(out=ct[:], in0=bi[:, s, D3:D3 + L],
                                                in1=crep[:, D3:D3 + L], op=ADD)
                        if s == 0:
                            eq = pdw.tile([128, L], F32, tag="eq")
                            nc.vector.tensor_tensor(
                                out=eq[:], in0=user_t[:],
                                in1=item_t[:, 0:1].to_broadcast([BSH, L]),
                                op=mybir.AluOpType.is_equal)
                            pen = pdw.tile([128, L], F32, tag="pen")
                            nc.vector.tensor_scalar_mul(pen[:], eq[:], PEN)
                            nc.vector.tensor_tensor(out=ct[:], in0=ct[:],
                                                    in1=pen[:], op=ADD)
                        nc.vector.tensor_tensor(out=ct[:], in0=s0b[:, s, :],
                                                in1=ct[:], op=ADD)
                        expa = pdw.tile([128, L], F32, tag="expa")
                        nc.scalar.activation(
                            expa[:], ct[:], mybir.ActivationFunctionType.Exp,
                            scale=float(1.0 / np.sqrt(D3)),
                            accum_out=denb[:, s:s + 1])
                        wu = pdw.tile([128, L], F32, tag="wu")
                        nc.vector.tensor_tensor(out=wu[:], in0=expa[:],
                                                in1=u0i[:, :, s], op=MUL)
                        nc.vector.tensor_reduce(num_all[:, s:s + 1], wu[:],
                                                axis=mybir.AxisListType.X, op=ADD)
                        nc.vector.tensor_copy(bvd_all[:, s:s + 1],
                                              bi[:, s, D3 + L:D3 + L + 1])

                    # device outputs [num + bvd*S | S]; host finishes
                    # pred = (num + bvd*S)/sqrt(S) -- keeps Exp as the only
                    # ACT table function (no per-rep table reloads)
                    t2 = pd.tile([128, 2], F32)
                    nc.vector.tensor_tensor(out=t2[:], in0=bvd_all[:],
                                            in1=denb[:], op=MUL)
                    preds4 = pd.tile([128, 4], F32)
                    nc.vector.tensor_tensor(out=preds4[:, 0:2], in0=num_all[:],
                                            in1=t2[:], op=ADD)
                    nc.vector.tensor_copy(preds4[:, 2:4], denb[:])
                    nc.sync.dma_start(pred_d[:], preds4[:])

            for rep in range(repeat):
                with tc.tile_pool(name=f"pard{rep}", bufs=1) as pardp:
                    par = {
                        "bi": pardp.tile([128, 2, TABB_W], BF,
                                         name=f"bi_{rep}"),
                    }
                    with tc.tile_pool(name=f"parc{rep}", bufs=1) as parcp:
                        par["rgT"] = parcp.tile([128, 2, NPAD], BF,
                                                name=f"rgT_{rep}")
                        par["regtg"] = parcp.tile([128, 2, D2], BF,
                                                  name=f"regtg_{rep}")
                        if "A" in phases:
                            _phase_a(rep, par)
                        if "C" in phases:
                            _phase_c(rep, par)
                    if "G" in phases:
                        _phase_g(rep, par)
                    if "D" in phases:
                        _phase_d(rep, par)

    nc.compile()
    return nc


def _prep_linear(inputs):
    f = np.float32
    user = np.asarray(inputs["user"]).astype(np.int32)
    item_i = np.asarray(inputs["item_i"]).astype(np.int32)
    item_j = np.asarray(inputs["item_j"]).astype(np.int32)
    emb_item = np.asarray(inputs["emb_item"], dtype=f)
    emb_in = np.asarray(inputs["emb_in"], dtype=f)
    emb_out = np.asarray(inputs["emb_out"], dtype=f)
    Wq = np.asarray(inputs["Wq"], dtype=f)
    bq = np.asarray(inputs["bq"], dtype=f)
    Wk = np.asarray(inputs["Wk"], dtype=f)
    bk = np.asarray(inputs["bk"], dtype=f)
    Wv = np.asarray(inputs["Wv"], dtype=f)
    bv = np.asarray(inputs["bv"], dtype=f)

    q = np.concatenate([emb_in, emb_out], 1)            # [N, 256]
    qt2 = np.zeros((D2, NPAD), f)
    qt2[:, :N_ITEMS] = q.T
    qt2 = qt2.reshape(2, 128, NPAD).astype(bf16)
    qe = np.zeros((NPAD, D2 + 1), f)
    qe[:N_ITEMS, :D2] = q
    qe[:N_ITEMS, D2] = 1.0
    qe = qe.reshape(NCH, 128, D2 + 1).astype(bf16)
    embT = np.zeros((128, NPAD), f)
    embT[:, :N_ITEMS] = emb_item.T
    embT = embT.astype(bf16)
    embg = emb_item.astype(bf16)

    lgrid, dgrid = np.meshgrid(np.arange(L), np.arange(D3), indexing="ij")
    BKp = bk[(100 * dgrid + lgrid) % D3].astype(f)      # [L, D3]
    WqT = Wq.T
    Wc = WqT @ BKp.T                                    # [384, 100]
    cq = bq @ BKp.T                                     # [100]
    WkT = Wk.T
    rhsK = np.stack([WkT[128 * j:128 * (j + 1)] for j in range(3)]).astype(bf16)
    rhsB = np.zeros((3, 128, TABB_W), f)
    for j in range(3):
        rhsB[j, :, 0:D3] = WqT[128 * j:128 * (j + 1)]
        rhsB[j, :, D3:D3 + L] = Wc[128 * j:128 * (j + 1)]
        rhsB[j, :, D3 + L] = bv[128 * j:128 * (j + 1)]
    rhsB = rhsB.astype(bf16)
    wv3 = np.stack([Wv[128 * j:128 * (j + 1)] for j in range(3)]).astype(bf16)
    consts = np.zeros((1, D3 + L), f)
    consts[0, :D3] = bq
    consts[0, D3:] = cq
    consts = consts.astype(bf16)

    sq = q.sum(0, dtype=np.float64).astype(f)           # [256]
    sqr = np.zeros((1, D2 + 1), f)
    sqr[0, :D2] = sq
    sqr[0, D2] = float(N_ITEMS)
    sqc = np.ascontiguousarray(sq.reshape(2, 128).T)    # [128, 2]

    shared = dict(qt2=qt2, qe=qe, embT=embT, embg=embg, rhsK=rhsK, rhsB=rhsB,
                  wv3=wv3, consts=consts, sqr=sqr, sqc=sqc)
    in_maps = []
    for c in range(NCORES):
        usr = user[c * BSH:(c + 1) * BSH]
        itm = np.stack([item_i[c * BSH:(c + 1) * BSH],
                        item_j[c * BSH:(c + 1) * BSH]], 1).astype(np.int32)
        m = dict(shared)
        m["user"] = np.ascontiguousarray(usr)
        # dma_gather index plane: logical idx i = l*BSH + b -> u[b, l],
        # laid out [16, n/16] (i%16, i//16), replicated to 8 Q7 stripes
        flat = usr.T.reshape(-1).astype(np.int16)          # i = l*128 + b
        idx16 = np.ascontiguousarray(flat.reshape(-1, 16).T)
        m["userg"] = np.tile(idx16, (8, 1))
        m["item"] = itm
        in_maps.append(m)
    return in_maps


def score_bound(inputs):
    """Rigorous |scores| bound via Cauchy-Schwarz: max||q|| * max||k|| / 16.
    q and k are column-swapped copies of each other, so the norms coincide."""
    emb_in = np.asarray(inputs["emb_in"], dtype=np.float32)
    emb_out = np.asarray(inputs["emb_out"], dtype=np.float32)
    nrm2 = (emb_in * emb_in).sum(1) + (emb_out * emb_out).sum(1)
    return float(nrm2.max()) / np.sqrt(2.0 * D)




def _build_exact(repeat=1, phases="ABCD"):
    import concourse.bass as bass
    import concourse.tile as tile
    from concourse import bacc, mybir
    from concourse.masks import make_identity

    F32 = mybir.dt.float32
    BF = mybir.dt.bfloat16
    I32 = mybir.dt.int32
    MUL = mybir.AluOpType.mult
    ADD = mybir.AluOpType.add

    nc = bacc.Bacc("TRN2", target_bir_lowering=False, debug=False,
                   num_devices=NCORES)

    def din(name, shape, dt):
        return nc.dram_tensor(name, shape, dt, kind="ExternalInput").ap()

    kt_d = din("kt", [2, 128, NPAD], BF)
    qt_d = din("qt", [2, 128, NSH], BF)
    qe_d = din("qe", [NCH, 128, D2 + 1], BF)
    embT_d = din("embT", [128, NPAD], BF)
    embg_d = din("embg", [N_ITEMS, D], BF)
    rhsK_d = din("rhsK", [3, 128, D3], BF)
    rhsB_d = din("rhsB", [3, 128, TABB_W], BF)
    wv3_d = din("wv3", [3, 128, D3], BF)
    consts_d = din("consts", [1, D3 + L], BF)
    user_d = din("user", [BSH, L], I32)
    item_d = din("item", [BSH, 2], I32)
    pred_d = nc.dram_tensor("pred", [BSH, 2], F32, kind="ExternalOutput").ap()

    with tile.TileContext(nc) as tc:
        with (
            tc.tile_pool(name="persist", bufs=1) as pp,
            tc.tile_pool(name="dram", bufs=1, space="DRAM") as dr,
        ):
            reg_sh = dr.tile([NSH, D2], BF)
            reg_full = dr.tile([N_ITEMS, D2], BF)
            tabKG = dr.tile([NPAD, D3 + D2], BF)
            tabB = dr.tile([NPAD, TABB_W], BF)

            # persistent small tiles
            ident = pp.tile([128, 128], BF)
            make_identity(nc, ident[:])
            user_t = pp.tile([BSH, L], I32)
            nc.sync.dma_start(user_t[:], user_d[:])
            item_t = pp.tile([BSH, 2], I32)
            nc.sync.dma_start(item_t[:], item_d[:])
            crow = pp.tile([1, D3 + L], BF)
            nc.sync.dma_start(crow[:], consts_d[:])
            crep = pp.tile([128, D3 + L], BF)
            nc.gpsimd.partition_broadcast(crep[:], crow[:])

            def _phase_a(rep):
                with (
                    tc.tile_pool(name=f"pa{rep}", bufs=1) as pa,
                    tc.tile_pool(name=f"pa_w{rep}", bufs=3) as pw,
                    tc.tile_pool(name=f"pa_ps{rep}", bufs=2, space="PSUM") as pps,
                    tc.tile_pool(name=f"pa_pr{rep}", bufs=1, space="PSUM") as ppr,
                ):
                    kt_sb = pa.tile([128, 2, NPAD], BF)
                    nc.sync.dma_start(kt_sb[:], kt_d[:].rearrange("c p n -> p c n"))
                    qt_sb = pa.tile([128, 2, NSH], BF)
                    nc.sync.dma_start(qt_sb[:], qt_d[:].rearrange("c p m -> p c m"))
                    qe_sb = pa.tile([128, NCH, D2 + 1], BF)
                    nc.sync.dma_start(qe_sb[:], qe_d[:].rearrange("c p w -> p c w"))

                    for m0, mbw in MBLOCKS:
                        nsub = (mbw + 127) // 128
                        psr = [ppr.tile([128, D2 + 1], F32, tag=f"psr{i}",
                                        name=f"psr{i}_{rep}")
                               for i in range(nsub)]
                        for ci in range(NCH):
                            psum_s = pps.tile([128, mbw], F32, tag="psum_s")
                            for kc in range(2):
                                nc.tensor.matmul(
                                    psum_s[:],
                                    kt_sb[:, kc, ci * 128:(ci + 1) * 128],
                                    qt_sb[:, kc, m0:m0 + mbw],
                                    start=(kc == 0), stop=(kc == 1))
                            e_sb = pw.tile([128, mbw], BF, tag="e_sb")
                            nc.scalar.activation(
                                e_sb[:], psum_s[:],
                                mybir.ActivationFunctionType.Exp, scale=1.0 / 16.0)
                            for si in range(nsub):
                                sw = min(128, mbw - si * 128)
                                nc.tensor.matmul(
                                    psr[si][:sw, :],
                                    e_sb[:, si * 128:si * 128 + sw],
                                    qe_sb[:, ci, :],
                                    start=(ci == 0), stop=(ci == NCH - 1))
                        for si in range(nsub):
                            r0 = m0 + si * 128
                            rows = min(128, NSH - r0)
                            rden = pw.tile([128, 1], F32, tag="rden")
                            nc.vector.reciprocal(rden[:rows], psr[si][:rows, D2:D2 + 1])
                            regmb = pw.tile([128, D2], BF, tag="regmb")
                            nc.vector.tensor_scalar_mul(
                                regmb[:rows], psr[si][:rows, 0:D2], rden[:rows])
                            nc.sync.dma_start(reg_sh[r0:r0 + rows, :], regmb[:rows])

            # ---------------- Phase B: AllGather region ----------------
            def _phase_b(rep):
                nc.gpsimd.collective_compute(
                    "AllGather", mybir.AluOpType.bypass,
                    replica_groups=[list(range(NCORES))],
                    ins=[reg_sh.opt()], outs=[reg_full.opt()])

            # ---------------- Phase C: tables ----------------
            def _phase_c(rep):
                with (
                    tc.tile_pool(name=f"pc{rep}", bufs=1) as pc,
                    tc.tile_pool(name=f"pc_w{rep}", bufs=3) as pcw,
                    tc.tile_pool(name=f"pc_ps{rep}", bufs=2, space="PSUM") as pcps,
                ):
                    et_sb = pc.tile([128, NPAD], BF)
                    nc.sync.dma_start(et_sb[:], embT_d[:])
                    rgT = pc.tile([128, 2, NPAD], BF)
                    nc.gpsimd.memset(rgT[:], 0.0)
                    for kc in range(2):
                        nc.sync.dma_start_transpose(
                            rgT[:, kc, 0:N_ITEMS],
                            reg_full[:, kc * 128:(kc + 1) * 128])
                    rk_sb = pc.tile([128, 3, D3], BF)
                    nc.sync.dma_start(rk_sb[:], rhsK_d[:].rearrange("c p w -> p c w"))
                    rb_sb = pc.tile([128, 3, TABB_W], BF)
                    nc.sync.dma_start(rb_sb[:], rhsB_d[:].rearrange("c p w -> p c w"))
                    wv_sb = pc.tile([128, 3, D3], BF)
                    nc.sync.dma_start(wv_sb[:], wv3_d[:].rearrange("c p w -> p c w"))

                    # targets: gather [emb | region] rows for item_i / item_j
                    tgt = pc.tile([128, 2, D3], BF)
                    for s in range(2):
                        nc.gpsimd.indirect_dma_start(
                            out=tgt[:, s, 0:D], out_offset=None, in_=embg_d[:],
                            in_offset=bass.IndirectOffsetOnAxis(
                                ap=item_t[:, s:s + 1], axis=0))
                        nc.gpsimd.indirect_dma_start(
                            out=tgt[:, s, D:D3], out_offset=None, in_=reg_full[:],
                            in_offset=bass.IndirectOffsetOnAxis(
                                ap=item_t[:, s:s + 1], axis=0))
                    # transpose targets -> tcT [feat, (pos128|neg128)]
                    tcT = pc.tile([128, 3, 2 * BSH], BF)
                    for oc in range(3):
                        for s in range(2):
                            pstr = pcps.tile([128, 128], BF, tag="pstr", bufs=1)
                            nc.tensor.transpose(
                                pstr[:], tgt[:, s, oc * 128:(oc + 1) * 128], ident[:])
                            nc.vector.tensor_copy(
                                tcT[:, oc, s * BSH:(s + 1) * BSH], pstr[:])
                    # M[in, tgt] = sum_out Wv[out, in] * tcT[out, tgt]
                    grhs = pc.tile([128, 3, 2 * BSH], BF)
                    for ic in range(3):
                        psM = pcps.tile([128, 2 * BSH], F32, tag="psM", bufs=1)
                        for oc in range(3):
                            nc.tensor.matmul(
                                psM[:], wv_sb[:, oc, ic * 128:(ic + 1) * 128],
                                tcT[:, oc, :], start=(oc == 0), stop=(oc == 2))
                        nc.vector.tensor_copy(grhs[:, ic, :], psM[:])

                    # table matmuls, 79 chunks of 128 items
                    for ch in range(NCH):
                        sl = slice(ch * 128, (ch + 1) * 128)
                        psK = pcps.tile([128, D3], F32, tag="psK")
                        psG = pcps.tile([128, D2], F32, tag="psG")
                        psB = pcps.tile([128, TABB_W], F32, tag="psB")
                        for j in range(3):
                            lh = et_sb[:, sl] if j == 0 else rgT[:, j - 1, sl]
                            nc.tensor.matmul(psK[:], lh, rk_sb[:, j, :],
                                             start=(j == 0), stop=(j == 2))
                            nc.tensor.matmul(psG[:], lh, grhs[:, j, :],
                                             start=(j == 0), stop=(j == 2))
                            nc.tensor.matmul(psB[:], lh, rb_sb[:, j, :],
                                             start=(j == 0), stop=(j == 2))
                        cKG = pcw.tile([128, D3 + D2], BF, tag="cKG")
                        nc.vector.tensor_copy(cKG[:, 0:D3], psK[:])
                        nc.scalar.copy(cKG[:, D3:D3 + D2], psG[:])
                        nc.sync.dma_start(tabKG[sl, :], cKG[:])
                        cB = pcw.tile([128, TABB_W], BF, tag="cB")
                        nc.vector.tensor_copy(cB[:], psB[:])
                        nc.sync.dma_start(tabB[sl, :], cB[:])

            # ---------------- Phase D: attention_network ----------------
            def _phase_d(rep):
                with (
                    tc.tile_pool(name=f"pd{rep}", bufs=1) as pd,
                    tc.tile_pool(name=f"pd_w{rep}", bufs=2) as pdw,
                ):
                    # gathers
                    ke = pd.tile([128, L, D3], BF)
                    bi = pd.tile([128, 2, TABB_W], BF)
                    for s in range(2):
                        nc.gpsimd.indirect_dma_start(
                            out=bi[:, s, :], out_offset=None, in_=tabB[:],
                            in_offset=bass.IndirectOffsetOnAxis(
                                ap=item_t[:, s:s + 1], axis=0))

                    preds = pd.tile([128, 2], F32)
                    ke_scr = ke[:].rearrange("p a b -> p (a b)").rearrange(
                        "p (d l) -> p l d", l=L)  # [128, L, D3] scrambled view

                    dens_all = pd.tile([128, 2], F32)
                    num_all = pd.tile([128, 2], F32)
                    bvd_all = pd.tile([128, 2], F32)

                    # merged K|G gather, l-chunked; diag-extract G in place
                    u0i = pd.tile([128, L, 2], F32)
                    LCH = 25
                    for l0 in range(0, L, LCH):
                        kg = pdw.tile([128, LCH, D3 + D2], BF, tag="kg", bufs=1)
                        for l in range(l0, l0 + LCH):
                            nc.gpsimd.indirect_dma_start(
                                out=kg[:, l - l0, :], out_offset=None,
                                in_=tabKG[:],
                                in_offset=bass.IndirectOffsetOnAxis(
                                    ap=user_t[:, l:l + 1], axis=0))
                        nc.scalar.copy(ke[:, l0:l0 + LCH, :], kg[:, :, 0:D3])
                        prod = pdw.tile([128, LCH, 2, BSH], BF, tag="prod")
                        nc.vector.tensor_tensor(
                            out=prod[:],
                            in0=kg[:, :, D3:D3 + D2].rearrange(
                                "p a (s t) -> p a s t", s=2),
                            in1=ident[:].unsqueeze(1).unsqueeze(1).to_broadcast(
                                [128, LCH, 2, BSH]),
                            op=MUL)
                        nc.vector.tensor_reduce(
                            u0i[:, l0:l0 + LCH, :].rearrange("p a b -> p (a b)"),
                            prod[:], axis=mybir.AxisListType.X, op=ADD)

                    for s in range(2):
                        qp = pdw.tile([128, D3], BF, tag="qp")
                        nc.vector.tensor_tensor(out=qp[:], in0=bi[:, s, 0:D3],
                                                in1=crep[:, 0:D3], op=ADD)
                        ct = pdw.tile([128, L], F32, tag="ct")
                        nc.vector.tensor_tensor(out=ct[:], in0=bi[:, s, D3:D3 + L],
                                                in1=crep[:, D3:D3 + L], op=ADD)
                        if s == 0:
                            eq = pdw.tile([128, L], F32, tag="eq")
                            nc.vector.tensor_tensor(
                                out=eq[:], in0=user_t[:],
                                in1=item_t[:, 0:1].to_broadcast([BSH, L]),
                                op=mybir.AluOpType.is_equal)
                            pen = pdw.tile([128, L], F32, tag="pen")
                            nc.vector.tensor_scalar_mul(pen[:], eq[:], PEN)
                            nc.vector.tensor_tensor(out=ct[:], in0=ct[:], in1=pen[:],
                                                    op=ADD)
                        # s0 via scrambled dot, l-chunked
                        s0 = pdw.tile([128, L], F32, tag="s0")
                        for l0 in range(0, L, LCH):
                            z = pdw.tile([128, LCH, D3], BF, tag="z")
                            nc.vector.tensor_tensor(
                                out=z[:], in0=ke_scr[:, l0:l0 + LCH, :],
                                in1=qp[:].unsqueeze(1).to_broadcast([128, LCH, D3]),
                                op=MUL)
                            nc.vector.tensor_reduce(
                                s0[:, l0:l0 + LCH], z[:],
                                axis=mybir.AxisListType.X, op=ADD)
                        nc.vector.tensor_tensor(out=s0[:], in0=s0[:], in1=ct[:], op=ADD)
                        expa = pdw.tile([128, L], F32, tag="expa")
                        den = pdw.tile([128, 1], F32, tag="den")
                        nc.scalar.activation(
                            expa[:], s0[:], mybir.ActivationFunctionType.Exp,
                            scale=float(1.0 / np.sqrt(D3)), accum_out=den[:])
                        nc.scalar.sqrt(dens_all[:, s:s + 1], den[:])
                        # num = sum_l expa * u0
                        wu = pdw.tile([128, L], F32, tag="wu")
                        nc.vector.tensor_tensor(out=wu[:], in0=expa[:],
                                                in1=u0i[:, :, s], op=MUL)
                        nc.vector.tensor_reduce(num_all[:, s:s + 1], wu[:],
                                                axis=mybir.AxisListType.X, op=ADD)
                        nc.vector.tensor_copy(bvd_all[:, s:s + 1],
                                              bi[:, s, D3 + L:D3 + L + 1])

                    # pred = num / dens + bvdot * dens
                    rdens = pd.tile([128, 2], F32)
                    nc.vector.reciprocal(rdens[:], dens_all[:])
                    t1 = pd.tile([128, 2], F32)
                    nc.vector.tensor_tensor(out=t1[:], in0=num_all[:], in1=rdens[:],
                                            op=MUL)
                    t2 = pd.tile([128, 2], F32)
                    nc.vector.tensor_tensor(out=t2[:], in0=bvd_all[:],
                                            in1=dens_all[:], op=MUL)
                    nc.vector.tensor_tensor(out=preds[:], in0=t1[:], in1=t2[:], op=ADD)
                    nc.sync.dma_start(pred_d[:], preds[:])

            for rep in range(repeat):
                if "A" in phases:
                    _phase_a(rep)
                if "B" in phases:
                    _phase_b(rep)
                if "C" in phases:
                    _phase_c(rep)
                if "D" in phases:
                    _phase_d(rep)

    nc.compile()
    return nc


def _prep_exact(inputs):
    f = np.float32
    user = np.asarray(inputs["user"]).astype(np.int32)
    item_i = np.asarray(inputs["item_i"]).astype(np.int32)
    item_j = np.asarray(inputs["item_j"]).astype(np.int32)
    emb_item = np.asarray(inputs["emb_item"], dtype=f)
    emb_in = np.asarray(inputs["emb_in"], dtype=f)
    emb_out = np.asarray(inputs["emb_out"], dtype=f)
    Wq = np.asarray(inputs["Wq"], dtype=f)
    bq = np.asarray(inputs["bq"], dtype=f)
    Wk = np.asarray(inputs["Wk"], dtype=f)
    bk = np.asarray(inputs["bk"], dtype=f)
    Wv = np.asarray(inputs["Wv"], dtype=f)
    bv = np.asarray(inputs["bv"], dtype=f)

    q = np.concatenate([emb_in, emb_out], 1)            # [N, 256]
    k = np.concatenate([emb_out, emb_in], 1)
    kT = np.zeros((D2, NPAD), f)
    kT[:, :N_ITEMS] = k.T
    kt = kT.reshape(2, 128, NPAD).astype(bf16)
    qT = np.ascontiguousarray(q.T)                      # [256, 10000]
    qe = np.zeros((NPAD, D2 + 1), f)
    qe[:N_ITEMS, :D2] = q
    qe[:N_ITEMS, D2] = 1.0
    qe = qe.reshape(NCH, 128, D2 + 1).astype(bf16)
    embT = np.zeros((128, NPAD), f)
    embT[:, :N_ITEMS] = emb_item.T
    embT = embT.astype(bf16)
    embg = emb_item.astype(bf16)

    lgrid, dgrid = np.meshgrid(np.arange(L), np.arange(D3), indexing="ij")
    BKp = bk[(100 * dgrid + lgrid) % D3].astype(f)      # [L, D3]
    WqT = Wq.T
    Wc = WqT @ BKp.T                                    # [384, 100]
    cq = bq @ BKp.T                                     # [100]
    WkT = Wk.T
    rhsK = np.stack([WkT[128 * j:128 * (j + 1)] for j in range(3)]).astype(bf16)
    rhsB = np.zeros((3, 128, TABB_W), f)
    for j in range(3):
        rhsB[j, :, 0:D3] = WqT[128 * j:128 * (j + 1)]
        rhsB[j, :, D3:D3 + L] = Wc[128 * j:128 * (j + 1)]
        rhsB[j, :, D3 + L] = bv[128 * j:128 * (j + 1)]
    rhsB = rhsB.astype(bf16)
    wv3 = np.stack([Wv[128 * j:128 * (j + 1)] for j in range(3)]).astype(bf16)
    consts = np.zeros((1, D3 + L), f)
    consts[0, :D3] = bq
    consts[0, D3:] = cq
    consts = consts.astype(bf16)

    shared = dict(kt=kt, qe=qe, embT=embT, embg=embg, rhsK=rhsK, rhsB=rhsB,
                  wv3=wv3, consts=consts)
    in_maps = []
    for c in range(NCORES):
        qts = np.ascontiguousarray(
            qT[:, c * NSH:(c + 1) * NSH]).reshape(2, 128, NSH).astype(bf16)
        usr = user[c * BSH:(c + 1) * BSH]
        itm = np.stack([item_i[c * BSH:(c + 1) * BSH],
                        item_j[c * BSH:(c + 1) * BSH]], 1).astype(np.int32)
        m = dict(shared)
        m["qt"] = qts
        m["user"] = np.ascontiguousarray(usr)
        m["item"] = itm
        in_maps.append(m)
    return in_maps




_CACHE = {}

# aliases used by test.py / bench tooling (linear path is the default)
def _build_program(repeat=1, phases="ACD"):
    return _build_linear(repeat=repeat, phases=phases)


def _prep_inputs(inputs):
    return _prep_linear(inputs)


def kernel(**inputs):
    from concourse.bass_utils import run_bass_kernel_spmd
    use_linear = score_bound(inputs) <= SCORE_BOUND
    key = "nc_lin" if use_linear else "nc_exact"
    if key not in _CACHE:
        _CACHE[key] = _build_linear() if use_linear else _build_exact()
    nc = _CACHE[key]
    in_maps = _prep_linear(inputs) if use_linear else _prep_exact(inputs)
    res = run_bass_kernel_spmd(nc, in_maps, list(range(NCORES))).results
    out = np.concatenate([res[c]["pred"] for c in range(NCORES)])
    if use_linear:  # device left [num + bvd*S | S]; finish pred here
        out = out[:, 0:2] / np.sqrt(out[:, 2:4])
    return (np.ascontiguousarray(out[:, 0], dtype=np.float32),
            np.ascontiguousarray(out[:, 1], dtype=np.float32))

